# revision 27
# baseline (speedup 1.0000x reference)
"""Trainium2 Bass kernel for MiniGPT4 CAM sparse attention.

Sharding: tensor-parallel over 16 heads -> 2 heads per core (8 cores).
Each core: q/k/v projections for its 2 heads (k/v only at the CAM-allowed
kv positions: [0,410) + [3072,4096)), RoPE, attention with the softmax
denominator folded into an augmented-V matmul (ones column), and a
row-sharded output projection producing a partial [2048, 4096] final^T.
Partials are summed on the host (gather/unshard step).

The three phases are interleaved into one pipeline: the CAM kv subtiles
are projected first, so attention for a query block starts as soon as its
q^T columns exist, and the output projection follows per query block.
Matmuls run in bf16 (full-rate PE); accumulation fp32 in PSUM.
"""

import math
import numpy as np

HID = 2048
HEADS = 16
HEAD_DIM = 128
SEQ = 4096
N_CORES = 8
HPC = HEADS // N_CORES          # heads per core = 2
DLOC = HPC * HEAD_DIM           # 256 local head dims
START = math.ceil(0.1 * SEQ)    # 410
RECENT = math.ceil(0.25 * SEQ)  # 1024
KV_HI = SEQ - RECENT            # 3072
NKV = START + RECENT            # 1434
SHIFT = 15.0                    # global exp shift (cancels in softmax)

P = 128
N_STILES = SEQ // P             # 32 seq subtiles
N_HTILES = HID // P             # 16 hidden tiles
QB = 512                        # q block (free dim of scores matmuls)
N_QB = SEQ // QB                # 8

# kv subtiles: (seq_subtile_index, rows_used) covering [0,410) + [3072,4096)
KV_STILES = [(0, 128), (1, 128), (2, 128), (3, 26)] + [(24 + i, 128) for i in range(8)]
N_KVT = len(KV_STILES)          # 12
KV_COL = np.cumsum([0] + [r for _, r in KV_STILES]).tolist()
NKV_PAD = 1440

_CACHE = {}


def _build_program(mm="bf16"):
    import concourse.bass as bass
    import concourse.bacc as bacc
    import concourse.mybir as mybir
    from concourse.tile import TileContext
    from contextlib import ExitStack

    fp32 = mybir.dt.float32
    bf16 = mybir.dt.bfloat16
    AF = mybir.ActivationFunctionType
    ALU = mybir.AluOpType

    nc = bacc.Bacc()

    hs_d = nc.declare_dram_parameter("hs", [SEQ, HID], fp32, isOutput=False)
    wq_d = nc.declare_dram_parameter("wq", [HID, DLOC], fp32, isOutput=False)
    wk_d = nc.declare_dram_parameter("wk", [HID, DLOC], fp32, isOutput=False)
    wv_d = nc.declare_dram_parameter("wv", [HID, DLOC], fp32, isOutput=False)
    wo_d = nc.declare_dram_parameter("wo", [DLOC, HID], fp32, isOutput=False)
    bq_d = nc.declare_dram_parameter("bq", [1, DLOC], fp32, isOutput=False)
    bk_d = nc.declare_dram_parameter("bk", [1, DLOC], fp32, isOutput=False)
    bv_d = nc.declare_dram_parameter("bv", [1, DLOC], fp32, isOutput=False)
    bo_d = nc.declare_dram_parameter("bo", [P, N_HTILES], fp32, isOutput=False)
    cq_d = nc.declare_dram_parameter("cq", [N_STILES, P, 64], fp32, isOutput=False)
    sq_d = nc.declare_dram_parameter("sq", [N_STILES, P, 64], fp32, isOutput=False)
    ck_d = nc.declare_dram_parameter("ck", [N_STILES, P, 64], fp32, isOutput=False)
    sk_d = nc.declare_dram_parameter("sk", [N_STILES, P, 64], fp32, isOutput=False)
    id_d = nc.declare_dram_parameter("ident", [P, P], fp32, isOutput=False)
    out_d = nc.declare_dram_parameter("pout", [HID, SEQ], fp32, isOutput=True)

    with TileContext(nc) as tc, ExitStack() as top:
        const = top.enter_context(tc.tile_pool(name="const", bufs=1))
        persist = top.enter_context(tc.tile_pool(name="persist", bufs=1))
        work = top.enter_context(tc.tile_pool(name="work", bufs=1))
        psum = top.enter_context(tc.tile_pool(name="psum", bufs=1, space="PSUM"))

        # ---- constants (bf16 matmul operands via SWDGE cast DMA) ----
        # fused [wq | wk] per h-tile so kv subtiles project q and k together
        wqk_sb = const.tile([P, N_HTILES, 2 * DLOC], bf16)
        nc.gpsimd.dma_start(
            wqk_sb[:].rearrange("p t (x d) -> p t x d", x=2)[:, :, 0, :],
            wq_d[:].rearrange("(t p) d -> p t d", p=P))
        nc.gpsimd.dma_start(
            wqk_sb[:].rearrange("p t (x d) -> p t x d", x=2)[:, :, 1, :],
            wk_d[:].rearrange("(t p) d -> p t d", p=P))
        wv_sb = const.tile([P, N_HTILES, DLOC], bf16)
        nc.gpsimd.dma_start(wv_sb[:], wv_d[:].rearrange("(t p) d -> p t d", p=P))
        wo_sb = const.tile([P, HPC, HID], bf16)
        nc.gpsimd.dma_start(wo_sb[:], wo_d[:].rearrange("(t p) e -> p t e", p=P))

        bqk_sb = const.tile([1, 2 * DLOC], bf16)
        nc.gpsimd.dma_start(bqk_sb[:, 0:DLOC], bq_d[:])
        nc.gpsimd.dma_start(bqk_sb[:, DLOC:2 * DLOC], bk_d[:])
        bv_sb = const.tile([1, DLOC], bf16)
        nc.gpsimd.dma_start(bv_sb[:], bv_d[:])
        boc_sb = const.tile([P, N_HTILES], fp32)
        nc.sync.dma_start(boc_sb[:], bo_d[:])
        ident_sb = const.tile([P, P], bf16)
        nc.gpsimd.dma_start(ident_sb[:], id_d[:])
        cq_sb = const.tile([P, N_STILES, 64], fp32)
        sq_sb = const.tile([P, N_STILES, 64], fp32)
        nc.sync.dma_start(cq_sb[:], cq_d[:].rearrange("t p d -> p t d"))
        nc.sync.dma_start(sq_sb[:], sq_d[:].rearrange("t p d -> p t d"))
        ones_sb = const.tile([1, QB], bf16)
        nc.gpsimd.memset(ones_sb[:], 1.0)
        onescol_sb = const.tile([P, 1], bf16)
        nc.gpsimd.memset(onescol_sb[:], 1.0)
        shift_sb = const.tile([P, 1], fp32)
        nc.gpsimd.memset(shift_sb[:], -SHIFT)

        # ---- persistent activations ----
        qT_sb = persist.tile([P, HPC, SEQ], bf16)
        kT_sb = persist.tile([P, HPC, NKV_PAD], bf16)
        vA_sb = persist.tile([P, HPC, N_KVT, P], bf16)
        oT_sb = persist.tile([P, HPC, SEQ], bf16)

        kv_of_stile = {st: (i, rows) for i, (st, rows) in enumerate(KV_STILES)}

        # ---------------- phase A worker: one seq subtile ----------------
        def do_subtile(st):
            is_kv = st in kv_of_stile
            hs_t = work.tile([P, HID], bf16, tag="hs", bufs=3, name=f"hs{st}")
            nc.gpsimd.dma_start(hs_t[:], hs_d[st * P:(st + 1) * P, :])

            hsT = work.tile([P, N_HTILES, P], bf16, tag="hsT", bufs=3,
                            name=f"hsT{st}")
            # transpose on the DMA xbar (bf16): SBUF->SBUF, no PE/PSUM cost
            for ht in range(N_HTILES):
                nc.sync.dma_start(hsT[:, ht, :], hs_t[:, ht * P:(ht + 1) * P],
                                  transpose=True)

            # q (and on kv subtiles also k) projection, fused rhs [wq|wk]
            nw = 2 * DLOC if is_kv else DLOC
            qkp = psum.tile([P, 2 * DLOC], fp32, tag="qp", bufs=2, name=f"qk{st}")
            nc.tensor.matmul(qkp[:, 0:nw], ones_sb[:, 0:P], bqk_sb[:, 0:nw],
                             start=True, stop=False)
            for ht in range(N_HTILES):
                nc.tensor.matmul(qkp[:, 0:nw], hsT[:, ht, :], wqk_sb[:, ht, 0:nw],
                                 start=False, stop=(ht == N_HTILES - 1))

            def rope(dst, src, cos_ap, sin_ap, rows=P):
                sr = src.rearrange("p (h x d) -> p h x d", h=HPC, x=2)
                dr = dst.rearrange("p (h x d) -> p h x d", h=HPC, x=2)
                x1, x2 = sr[:, :, 0, :], sr[:, :, 1, :]
                o1, o2 = dr[:, :, 0, :], dr[:, :, 1, :]
                cb = cos_ap.rearrange("p (o d) -> p o d", o=1).broadcast_to([rows, HPC, 64])
                sbb = sin_ap.rearrange("p (o d) -> p o d", o=1).broadcast_to([rows, HPC, 64])
                tmp = work.tile([P, P], fp32, tag="rtmp", bufs=3, name=f"rt{st}")
                tr = tmp[0:rows, :].rearrange("p (h d) -> p h d", h=HPC)
                nc.vector.tensor_tensor(o1, x1, cb, ALU.mult)
                nc.vector.tensor_tensor(tr, x2, sbb, ALU.mult)
                nc.vector.tensor_tensor(o1, o1, tr, ALU.subtract)
                nc.vector.tensor_tensor(o2, x1, sbb, ALU.mult)
                nc.vector.tensor_tensor(tr, x2, cb, ALU.mult)
                nc.vector.tensor_tensor(o2, o2, tr, ALU.add)

            qr = work.tile([P, DLOC], bf16, tag="qr", bufs=3, name=f"qr{st}")
            rope(qr[:], qkp[:, 0:DLOC], cq_sb[:, st, :], sq_sb[:, st, :])
            # transpose roped q into qT (both heads -> one psum, one copy)
            pq = psum.tile([P, 2 * P], bf16, tag="tq", bufs=1, name=f"pq{st}")
            for h in range(HPC):
                nc.tensor.transpose(pq[:, h * P:(h + 1) * P],
                                    qr[:, h * P:(h + 1) * P], ident_sb[:])
            nc.vector.tensor_copy(
                qT_sb[:, :, st * P:(st + 1) * P],
                pq[:].rearrange("p (a b) -> p a b", a=HPC))

            if is_kv:
                ti, rows = kv_of_stile[st]
                col = KV_COL[ti]
                ck_t = work.tile([P, 64], fp32, tag="ck", bufs=2, name=f"ck{st}")
                sk_t = work.tile([P, 64], fp32, tag="sk", bufs=2, name=f"sk{st}")
                nc.sync.dma_start(ck_t[:rows, :], ck_d[st, 0:rows, :])
                nc.sync.dma_start(sk_t[:rows, :], sk_d[st, 0:rows, :])

                kr = work.tile([P, DLOC], bf16, tag="qr", bufs=3, name=f"kr{st}")
                rope(kr[0:rows, :], qkp[0:rows, DLOC:2 * DLOC],
                     ck_t[0:rows, :], sk_t[0:rows, :], rows=rows)
                for h in range(HPC):
                    pk = psum.tile([P, P], bf16, tag="tq", bufs=1,
                                   name=f"pk{st}_{h}")
                    nc.tensor.transpose(pk[:, 0:rows],
                                        kr[0:rows, h * P:(h + 1) * P],
                                        ident_sb[0:rows, 0:rows])
                    nc.vector.tensor_copy(kT_sb[:, h, col:col + rows],
                                          pk[:, 0:rows])

                vp = psum.tile([P, 2 * DLOC], fp32, tag="qp", bufs=2, name=f"v{st}")
                nc.tensor.matmul(vp[0:rows, 0:DLOC], ones_sb[:, 0:rows], bv_sb[:],
                                 start=True, stop=False)
                for ht in range(N_HTILES):
                    nc.tensor.matmul(vp[0:rows, 0:DLOC], hsT[:, ht, 0:rows],
                                     wv_sb[:, ht, :],
                                     start=False, stop=(ht == N_HTILES - 1))
                for h in range(HPC):
                    nc.vector.tensor_copy(vA_sb[0:rows, h, ti, 0:P],
                                          vp[0:rows, h * P:(h + 1) * P])

        # ---------------- phase B workers: attention for one q block ------
        ex_tiles = {}

        def do_scores(qb):
            for h in range(HPC):
                ex = work.tile([P, N_KVT, QB], bf16, tag="exp", bufs=4,
                               name=f"ex{qb}_{h}")
                ex_tiles[(qb, h)] = ex
                for ti, (st, rows) in enumerate(KV_STILES):
                    col = KV_COL[ti]
                    sp = psum.tile([P, QB], fp32, tag="sc", bufs=2,
                                   name=f"sc{qb}_{h}_{ti}")
                    nc.tensor.matmul(sp[0:rows, :],
                                     kT_sb[:, h, col:col + rows],
                                     qT_sb[:, h, qb * QB:(qb + 1) * QB],
                                     start=True, stop=True)
                    nc.scalar.activation(ex[0:rows, ti, :], sp[0:rows, :],
                                         AF.Exp, bias=shift_sb[0:rows, :],
                                         scale=1.0)

        # kv tiles with full 128 rows, for the denominator pair-tree
        FULL_TIS = [ti for ti, (st, r) in enumerate(KV_STILES) if r == P]
        SHORT_TI = [ti for ti, (st, r) in enumerate(KV_STILES) if r != P][0]

        def do_attnv(qb):
            for h in range(HPC):
                ex = ex_tiles.pop((qb, h))
                # O^T accumulation [128d, 512q] with N=512 matmuls
                op = psum.tile([P, QB], fp32, tag="ov", bufs=2,
                               name=f"ov{qb}_{h}")
                dn = psum.tile([1, QB], fp32, tag="ov", bufs=2,
                               name=f"dn{qb}_{h}")
                for ti, (st, rows) in enumerate(KV_STILES):
                    nc.tensor.matmul(op[:],
                                     vA_sb[0:rows, h, ti, :],
                                     ex[0:rows, ti, :],
                                     start=(ti == 0), stop=(ti == N_KVT - 1))
                    nc.tensor.matmul(dn[:],
                                     onescol_sb[0:rows, :],
                                     ex[0:rows, ti, :],
                                     start=(ti == 0), stop=(ti == N_KVT - 1))
                recip = work.tile([1, QB], fp32, tag="recip", bufs=3,
                                  name=f"rc{qb}_{h}")
                nc.vector.reciprocal(recip[:], dn[:])
                rb = work.tile([P, QB], fp32, tag="rb", bufs=3,
                               name=f"rb{qb}_{h}")
                nc.gpsimd.partition_broadcast(rb[:], recip[:])
                nc.vector.tensor_tensor(oT_sb[:, h, qb * QB:(qb + 1) * QB],
                                        op[:], rb[:], ALU.mult)

        # ---------------- phase C worker: out-proj for one q block --------
        def do_oproj(qb):
            for et in range(N_HTILES):
                fp = psum.tile([P, QB], fp32, tag="qp", bufs=2,
                               name=f"fp{qb}_{et}")
                for h in range(HPC):
                    nc.tensor.matmul(fp[:],
                                     wo_sb[:, h, et * P:(et + 1) * P],
                                     oT_sb[:, h, qb * QB:(qb + 1) * QB],
                                     start=(h == 0), stop=(h == HPC - 1))
                stg = work.tile([P, QB], fp32, tag="stage", bufs=4,
                                name=f"st{qb}_{et}")
                # copy PSUM->SBUF with the (per-partition) bo bias folded in
                nc.vector.tensor_scalar_add(stg[:], fp[:],
                                            boc_sb[:, et:et + 1])
                nc.sync.dma_start(
                    out_d[et * P:(et + 1) * P, qb * QB:(qb + 1) * QB], stg[:])

        # ---------------- interleaved pipeline ----------------
        # kv subtiles first; then per q block: scores -> (weave in remaining
        # subtiles so the PE never head-of-line-blocks on ACT's exp) ->
        # attention@V -> more subtiles -> out-projection.
        from collections import deque
        for st in [st for st, _ in KV_STILES]:
            do_subtile(st)
        pending = deque(range(4, 24))
        emitted = set(st for st, _ in KV_STILES)

        def emit_a(n):
            for _ in range(n):
                if pending:
                    st = pending.popleft()
                    do_subtile(st)
                    emitted.add(st)

        for qb in (0, 6, 7, 1, 2, 3, 4, 5):
            while not all(s in emitted for s in range(qb * 4, qb * 4 + 4)):
                emit_a(1)
            do_scores(qb)
            emit_a(2)
            do_attnv(qb)
            emit_a(1)
            do_oproj(qb)

    nc.finalize()
    return nc


def _host_inputs(inputs):
    hs = np.ascontiguousarray(np.asarray(inputs["hidden_states"], np.float32).reshape(SEQ, HID))
    Wq = np.asarray(inputs["Wq"], np.float32)
    Wk = np.asarray(inputs["Wk"], np.float32)
    Wv = np.asarray(inputs["Wv"], np.float32)
    Wo = np.asarray(inputs["Wo"], np.float32)
    bq = np.asarray(inputs["bq"], np.float32)
    bk = np.asarray(inputs["bk"], np.float32)
    bv = np.asarray(inputs["bv"], np.float32)
    bo = np.asarray(inputs["bo"], np.float32)

    theta = 1.0 / (10000.0 ** (np.arange(0, HEAD_DIM, 2, dtype=np.float32) / HEAD_DIM))
    sinusoid = np.arange(SEQ, dtype=np.float32)[:, None] * theta[None, :]
    sin = np.sin(sinusoid).astype(np.float32)
    cos = np.cos(sinusoid).astype(np.float32)
    scale = np.float32(1.0 / math.sqrt(HEAD_DIM))
    cq = (cos * scale).reshape(N_STILES, P, 64)
    sq = (sin * scale).reshape(N_STILES, P, 64)
    ck = cos.reshape(N_STILES, P, 64)
    sk = sin.reshape(N_STILES, P, 64)
    ident = np.eye(P, dtype=np.float32)

    perm = np.concatenate([np.arange(0, HEAD_DIM, 2), np.arange(1, HEAD_DIM, 2)])
    in_maps = []
    for c in range(N_CORES):
        cols_pk = np.concatenate([c * 256 + h * 128 + perm for h in range(HPC)])
        sl = slice(c * 256, (c + 1) * 256)
        in_maps.append({
            "hs": hs,
            "wq": np.ascontiguousarray(Wq.T[:, cols_pk]),
            "wk": np.ascontiguousarray(Wk.T[:, cols_pk]),
            "wv": np.ascontiguousarray(Wv.T[:, sl]),
            "wo": np.ascontiguousarray(Wo.T[sl, :]),
            "bq": np.ascontiguousarray(bq[cols_pk]).reshape(1, DLOC),
            "bk": np.ascontiguousarray(bk[cols_pk]).reshape(1, DLOC),
            "bv": np.ascontiguousarray(bv[sl]).reshape(1, DLOC),
            "bo": np.ascontiguousarray((bo / N_CORES).reshape(N_HTILES, P).T),
            "cq": cq, "sq": sq, "ck": ck, "sk": sk,
            "ident": ident,
        })
    return in_maps


def _maybe_enable_ldw_opt():
    """Experimental: the stock compile pipeline passes --enable-ldw-opt=false;
    flipping it on lets walrus pipeline LDWEIGHTS behind matmuls. Controlled
    by LDW_OPT=1 env; correctness is re-checked by the caller."""
    import os
    if os.environ.get("LDW_OPT", "0") != "1":
        return
    import concourse.bass_utils as bu
    if getattr(bu, "_ldw_patched", False):
        return
    orig = bu.bir_verify_and_optimise

    def patched(tmpdir, inp="bir.json", outp="file.neff", arch=None, **kw):
        import unittest.mock as um
        real_run = bu.run_command

        def run2(argv, **kwargs):
            argv = [a.replace("--enable-ldw-opt=false", "--enable-ldw-opt=true")
                    for a in argv]
            return real_run(argv, **kwargs)

        with um.patch.object(bu, "run_command", run2):
            return orig(tmpdir, inp, outp, arch, **kw)

    bu.bir_verify_and_optimise = patched
    # bass2jax binds its own reference
    import concourse.bass2jax as b2j
    if hasattr(b2j, "bir_verify_and_optimise"):
        b2j.bir_verify_and_optimise = patched
    bu._ldw_patched = True


def run(inputs, trace=False, mm="bf16"):
    _maybe_enable_ldw_opt()
    from concourse.bass_utils import run_bass_kernel_spmd
    key = mm
    if key not in _CACHE:
        _CACHE[key] = _build_program(mm)
    nc = _CACHE[key]
    in_maps = _host_inputs(inputs)
    res = run_bass_kernel_spmd(nc, in_maps, core_ids=list(range(N_CORES)),
                               trace=trace)
    acc = np.zeros((HID, SEQ), np.float64)
    for r in res.results:
        acc += r["pout"].astype(np.float64)
    out = acc.T.astype(np.float32).reshape(1, SEQ, HID)
    return out, res


def kernel(**inputs) -> np.ndarray:
    out, _ = run(inputs, trace=False)
    return out


# revision 28
# speedup vs baseline: 2.0580x; 2.0580x over previous
"""Trainium2 Bass kernel for MiniGPT4 CAM sparse attention.

Sharding: tensor-parallel over 16 heads -> 2 heads per core (8 cores).
Each core: q/k/v projections for its 2 heads (k/v only at the CAM-allowed
kv positions: [0,410) + [3072,4096)), RoPE, attention with the softmax
denominator folded into an augmented-V matmul (ones column), and a
row-sharded output projection producing a partial [2048, 4096] final^T.
Partials are summed on the host (gather/unshard step).

The three phases are interleaved into one pipeline: the CAM kv subtiles
are projected first, so attention for a query block starts as soon as its
q^T columns exist, and the output projection follows per query block.
Matmuls run in bf16 (full-rate PE); accumulation fp32 in PSUM.
"""

import math
import numpy as np

HID = 2048
HEADS = 16
HEAD_DIM = 128
SEQ = 4096
N_CORES = 8
HPC = HEADS // N_CORES          # heads per core = 2
DLOC = HPC * HEAD_DIM           # 256 local head dims
START = math.ceil(0.1 * SEQ)    # 410
RECENT = math.ceil(0.25 * SEQ)  # 1024
KV_HI = SEQ - RECENT            # 3072
NKV = START + RECENT            # 1434
SHIFT = 15.0                    # global exp shift (cancels in softmax)

P = 128
N_STILES = SEQ // P             # 32 seq subtiles
N_HTILES = HID // P             # 16 hidden tiles
QB = 512                        # q block (free dim of scores matmuls)
N_QB = SEQ // QB                # 8

# kv subtiles: (seq_subtile_index, rows_used) covering [0,410) + [3072,4096)
KV_STILES = [(0, 128), (1, 128), (2, 128), (3, 26)] + [(24 + i, 128) for i in range(8)]
N_KVT = len(KV_STILES)          # 12
KV_COL = np.cumsum([0] + [r for _, r in KV_STILES]).tolist()
NKV_PAD = 1440

_CACHE = {}


def _build_program(mm="bf16"):
    import concourse.bass as bass
    import concourse.bacc as bacc
    import concourse.mybir as mybir
    from concourse.tile import TileContext
    from contextlib import ExitStack

    fp32 = mybir.dt.float32
    bf16 = mybir.dt.bfloat16
    AF = mybir.ActivationFunctionType
    ALU = mybir.AluOpType

    nc = bacc.Bacc()

    hs_d = nc.declare_dram_parameter("hs", [SEQ, HID], fp32, isOutput=False)
    wq_d = nc.declare_dram_parameter("wq", [HID, DLOC], fp32, isOutput=False)
    wk_d = nc.declare_dram_parameter("wk", [HID, DLOC], fp32, isOutput=False)
    wv_d = nc.declare_dram_parameter("wv", [HID, DLOC], fp32, isOutput=False)
    wo_d = nc.declare_dram_parameter("wo", [DLOC, HID], fp32, isOutput=False)
    bq_d = nc.declare_dram_parameter("bq", [1, DLOC], fp32, isOutput=False)
    bk_d = nc.declare_dram_parameter("bk", [1, DLOC], fp32, isOutput=False)
    bv_d = nc.declare_dram_parameter("bv", [1, DLOC], fp32, isOutput=False)
    bo_d = nc.declare_dram_parameter("bo", [P, N_HTILES], fp32, isOutput=False)
    cq_d = nc.declare_dram_parameter("cq", [N_STILES, P, 64], fp32, isOutput=False)
    sq_d = nc.declare_dram_parameter("sq", [N_STILES, P, 64], fp32, isOutput=False)
    ck_d = nc.declare_dram_parameter("ck", [N_STILES, P, 64], fp32, isOutput=False)
    sk_d = nc.declare_dram_parameter("sk", [N_STILES, P, 64], fp32, isOutput=False)
    id_d = nc.declare_dram_parameter("ident", [P, P], fp32, isOutput=False)
    out_d = nc.declare_dram_parameter("pout", [HID, SEQ], fp32, isOutput=True)

    with TileContext(nc) as tc, ExitStack() as top:
        const = top.enter_context(tc.tile_pool(name="const", bufs=1))
        persist = top.enter_context(tc.tile_pool(name="persist", bufs=1))
        work = top.enter_context(tc.tile_pool(name="work", bufs=1))
        psum = top.enter_context(tc.tile_pool(name="psum", bufs=1, space="PSUM"))

        # ---- constants (bf16 matmul operands via SWDGE cast DMA) ----
        # fused [wq | wk] per h-tile so kv subtiles project q and k together
        wqk_sb = const.tile([P, N_HTILES, 2 * DLOC], bf16)
        nc.gpsimd.dma_start(
            wqk_sb[:].rearrange("p t (x d) -> p t x d", x=2)[:, :, 0, :],
            wq_d[:].rearrange("(t p) d -> p t d", p=P))
        nc.gpsimd.dma_start(
            wqk_sb[:].rearrange("p t (x d) -> p t x d", x=2)[:, :, 1, :],
            wk_d[:].rearrange("(t p) d -> p t d", p=P))
        wv_sb = const.tile([P, N_HTILES, DLOC], bf16)
        nc.gpsimd.dma_start(wv_sb[:], wv_d[:].rearrange("(t p) d -> p t d", p=P))
        wo_sb = const.tile([P, HPC, HID], bf16)
        nc.gpsimd.dma_start(wo_sb[:], wo_d[:].rearrange("(t p) e -> p t e", p=P))

        bqk_sb = const.tile([1, 2 * DLOC], bf16)
        nc.gpsimd.dma_start(bqk_sb[:, 0:DLOC], bq_d[:])
        nc.gpsimd.dma_start(bqk_sb[:, DLOC:2 * DLOC], bk_d[:])
        bv_sb = const.tile([1, DLOC], bf16)
        nc.gpsimd.dma_start(bv_sb[:], bv_d[:])
        boc_sb = const.tile([P, N_HTILES], fp32)
        nc.sync.dma_start(boc_sb[:], bo_d[:])
        ident_sb = const.tile([P, P], bf16)
        nc.gpsimd.dma_start(ident_sb[:], id_d[:])
        cq_sb = const.tile([P, N_STILES, 64], fp32)
        sq_sb = const.tile([P, N_STILES, 64], fp32)
        nc.sync.dma_start(cq_sb[:], cq_d[:].rearrange("t p d -> p t d"))
        nc.sync.dma_start(sq_sb[:], sq_d[:].rearrange("t p d -> p t d"))
        ones_sb = const.tile([1, QB], bf16)
        nc.gpsimd.memset(ones_sb[:], 1.0)
        onescol_sb = const.tile([P, 1], bf16)
        nc.gpsimd.memset(onescol_sb[:], 1.0)
        shift_sb = const.tile([P, 1], fp32)
        nc.gpsimd.memset(shift_sb[:], -SHIFT)

        # ---- persistent activations ----
        qT_sb = persist.tile([P, HPC, SEQ], bf16)
        kT_sb = persist.tile([P, HPC, NKV_PAD], bf16)
        vA_sb = persist.tile([P, HPC, N_KVT, P], bf16)
        oT_sb = persist.tile([P, HPC, SEQ], bf16)

        kv_of_stile = {st: (i, rows) for i, (st, rows) in enumerate(KV_STILES)}

        # ---------------- phase A worker: one seq subtile ----------------
        def do_subtile(st):
            is_kv = st in kv_of_stile
            hs_t = work.tile([P, HID], bf16, tag="hs", bufs=3, name=f"hs{st}")
            nc.gpsimd.dma_start(hs_t[:], hs_d[st * P:(st + 1) * P, :])

            hsT = work.tile([P, N_HTILES, P], bf16, tag="hsT", bufs=3,
                            name=f"hsT{st}")
            for g in range(N_HTILES // 4):
                ps = psum.tile([P, 4 * P], bf16, tag="tp", bufs=1, name=f"tp{st}_{g}")
                for j in range(4):
                    ht = g * 4 + j
                    nc.tensor.transpose(ps[:, j * P:(j + 1) * P],
                                        hs_t[:, ht * P:(ht + 1) * P], ident_sb[:])
                nc.vector.tensor_copy(
                    hsT[:, g * 4:(g + 1) * 4, :].rearrange("p a b -> p (a b)"),
                    ps[:])

            # q (and on kv subtiles also k) projection, fused rhs [wq|wk]
            nw = 2 * DLOC if is_kv else DLOC
            qkp = psum.tile([P, 2 * DLOC], fp32, tag="qp", bufs=2, name=f"qk{st}")
            nc.tensor.matmul(qkp[:, 0:nw], ones_sb[:, 0:P], bqk_sb[:, 0:nw],
                             start=True, stop=False)
            for ht in range(N_HTILES):
                nc.tensor.matmul(qkp[:, 0:nw], hsT[:, ht, :], wqk_sb[:, ht, 0:nw],
                                 start=False, stop=(ht == N_HTILES - 1))

            def rope(dst, src, cos_ap, sin_ap, rows=P):
                sr = src.rearrange("p (h x d) -> p h x d", h=HPC, x=2)
                dr = dst.rearrange("p (h x d) -> p h x d", h=HPC, x=2)
                x1, x2 = sr[:, :, 0, :], sr[:, :, 1, :]
                o1, o2 = dr[:, :, 0, :], dr[:, :, 1, :]
                cb = cos_ap.rearrange("p (o d) -> p o d", o=1).broadcast_to([rows, HPC, 64])
                sbb = sin_ap.rearrange("p (o d) -> p o d", o=1).broadcast_to([rows, HPC, 64])
                tmp = work.tile([P, P], fp32, tag="rtmp", bufs=3, name=f"rt{st}")
                tr = tmp[0:rows, :].rearrange("p (h d) -> p h d", h=HPC)
                nc.vector.tensor_tensor(o1, x1, cb, ALU.mult)
                nc.vector.tensor_tensor(tr, x2, sbb, ALU.mult)
                nc.vector.tensor_tensor(o1, o1, tr, ALU.subtract)
                nc.vector.tensor_tensor(o2, x1, sbb, ALU.mult)
                nc.vector.tensor_tensor(tr, x2, cb, ALU.mult)
                nc.vector.tensor_tensor(o2, o2, tr, ALU.add)

            qr = work.tile([P, DLOC], bf16, tag="qr", bufs=3, name=f"qr{st}")
            rope(qr[:], qkp[:, 0:DLOC], cq_sb[:, st, :], sq_sb[:, st, :])
            # transpose roped q into qT (both heads -> one psum, one copy)
            pq = psum.tile([P, 2 * P], bf16, tag="tq", bufs=1, name=f"pq{st}")
            for h in range(HPC):
                nc.tensor.transpose(pq[:, h * P:(h + 1) * P],
                                    qr[:, h * P:(h + 1) * P], ident_sb[:])
            nc.vector.tensor_copy(
                qT_sb[:, :, st * P:(st + 1) * P],
                pq[:].rearrange("p (a b) -> p a b", a=HPC))

            if is_kv:
                ti, rows = kv_of_stile[st]
                col = KV_COL[ti]
                ck_t = work.tile([P, 64], fp32, tag="ck", bufs=2, name=f"ck{st}")
                sk_t = work.tile([P, 64], fp32, tag="sk", bufs=2, name=f"sk{st}")
                nc.sync.dma_start(ck_t[:rows, :], ck_d[st, 0:rows, :])
                nc.sync.dma_start(sk_t[:rows, :], sk_d[st, 0:rows, :])

                kr = work.tile([P, DLOC], bf16, tag="qr", bufs=3, name=f"kr{st}")
                rope(kr[0:rows, :], qkp[0:rows, DLOC:2 * DLOC],
                     ck_t[0:rows, :], sk_t[0:rows, :], rows=rows)
                for h in range(HPC):
                    pk = psum.tile([P, P], bf16, tag="tq", bufs=1,
                                   name=f"pk{st}_{h}")
                    nc.tensor.transpose(pk[:, 0:rows],
                                        kr[0:rows, h * P:(h + 1) * P],
                                        ident_sb[0:rows, 0:rows])
                    nc.vector.tensor_copy(kT_sb[:, h, col:col + rows],
                                          pk[:, 0:rows])

                vp = psum.tile([P, 2 * DLOC], fp32, tag="qp", bufs=2, name=f"v{st}")
                nc.tensor.matmul(vp[0:rows, 0:DLOC], ones_sb[:, 0:rows], bv_sb[:],
                                 start=True, stop=False)
                for ht in range(N_HTILES):
                    nc.tensor.matmul(vp[0:rows, 0:DLOC], hsT[:, ht, 0:rows],
                                     wv_sb[:, ht, :],
                                     start=False, stop=(ht == N_HTILES - 1))
                for h in range(HPC):
                    nc.vector.tensor_copy(vA_sb[0:rows, h, ti, 0:P],
                                          vp[0:rows, h * P:(h + 1) * P])

        # ---------------- phase B workers: attention for one q block ------
        ex_tiles = {}

        def do_scores(qb):
            for h in range(HPC):
                ex = work.tile([P, N_KVT, QB], bf16, tag="exp", bufs=4,
                               name=f"ex{qb}_{h}")
                ex_tiles[(qb, h)] = ex
                for ti, (st, rows) in enumerate(KV_STILES):
                    col = KV_COL[ti]
                    sp = psum.tile([P, QB], fp32, tag="sc", bufs=2,
                                   name=f"sc{qb}_{h}_{ti}")
                    nc.tensor.matmul(sp[0:rows, :],
                                     kT_sb[:, h, col:col + rows],
                                     qT_sb[:, h, qb * QB:(qb + 1) * QB],
                                     start=True, stop=True)
                    nc.scalar.activation(ex[0:rows, ti, :], sp[0:rows, :],
                                         AF.Exp, bias=shift_sb[0:rows, :],
                                         scale=1.0)

        # kv tiles with full 128 rows, for the denominator pair-tree
        FULL_TIS = [ti for ti, (st, r) in enumerate(KV_STILES) if r == P]
        SHORT_TI = [ti for ti, (st, r) in enumerate(KV_STILES) if r != P][0]

        def do_attnv(qb):
            for h in range(HPC):
                ex = ex_tiles.pop((qb, h))
                # O^T accumulation [128d, 512q] with N=512 matmuls
                op = psum.tile([P, QB], fp32, tag="ov", bufs=2,
                               name=f"ov{qb}_{h}")
                dn = psum.tile([1, QB], fp32, tag="ov", bufs=2,
                               name=f"dn{qb}_{h}")
                for ti, (st, rows) in enumerate(KV_STILES):
                    nc.tensor.matmul(op[:],
                                     vA_sb[0:rows, h, ti, :],
                                     ex[0:rows, ti, :],
                                     start=(ti == 0), stop=(ti == N_KVT - 1))
                    nc.tensor.matmul(dn[:],
                                     onescol_sb[0:rows, :],
                                     ex[0:rows, ti, :],
                                     start=(ti == 0), stop=(ti == N_KVT - 1))
                recip = work.tile([1, QB], fp32, tag="recip", bufs=3,
                                  name=f"rc{qb}_{h}")
                nc.vector.reciprocal(recip[:], dn[:])
                rb = work.tile([P, QB], fp32, tag="rb", bufs=3,
                               name=f"rb{qb}_{h}")
                nc.gpsimd.partition_broadcast(rb[:], recip[:])
                nc.vector.tensor_tensor(oT_sb[:, h, qb * QB:(qb + 1) * QB],
                                        op[:], rb[:], ALU.mult)

        # ---------------- phase C worker: out-proj for one q block --------
        def do_oproj(qb):
            for et in range(N_HTILES):
                fp = psum.tile([P, QB], fp32, tag="qp", bufs=2,
                               name=f"fp{qb}_{et}")
                for h in range(HPC):
                    nc.tensor.matmul(fp[:],
                                     wo_sb[:, h, et * P:(et + 1) * P],
                                     oT_sb[:, h, qb * QB:(qb + 1) * QB],
                                     start=(h == 0), stop=(h == HPC - 1))
                stg = work.tile([P, QB], fp32, tag="stage", bufs=4,
                                name=f"st{qb}_{et}")
                # copy PSUM->SBUF with the (per-partition) bo bias folded in
                nc.vector.tensor_scalar_add(stg[:], fp[:],
                                            boc_sb[:, et:et + 1])
                nc.sync.dma_start(
                    out_d[et * P:(et + 1) * P, qb * QB:(qb + 1) * QB], stg[:])

        # ---------------- interleaved pipeline ----------------
        # kv subtiles first; then per q block: scores -> (weave in remaining
        # subtiles so the PE never head-of-line-blocks on ACT's exp) ->
        # attention@V -> more subtiles -> out-projection.
        from collections import deque
        for st in [st for st, _ in KV_STILES]:
            do_subtile(st)
        pending = deque(range(4, 24))
        emitted = set(st for st, _ in KV_STILES)

        def emit_a(n):
            for _ in range(n):
                if pending:
                    st = pending.popleft()
                    do_subtile(st)
                    emitted.add(st)

        for qb in (0, 6, 7, 1, 2, 3, 4, 5):
            while not all(s in emitted for s in range(qb * 4, qb * 4 + 4)):
                emit_a(1)
            do_scores(qb)
            emit_a(2)
            do_attnv(qb)
            emit_a(1)
            do_oproj(qb)

    nc.finalize()
    return nc


def _host_inputs(inputs):
    hs = np.ascontiguousarray(np.asarray(inputs["hidden_states"], np.float32).reshape(SEQ, HID))
    Wq = np.asarray(inputs["Wq"], np.float32)
    Wk = np.asarray(inputs["Wk"], np.float32)
    Wv = np.asarray(inputs["Wv"], np.float32)
    Wo = np.asarray(inputs["Wo"], np.float32)
    bq = np.asarray(inputs["bq"], np.float32)
    bk = np.asarray(inputs["bk"], np.float32)
    bv = np.asarray(inputs["bv"], np.float32)
    bo = np.asarray(inputs["bo"], np.float32)

    theta = 1.0 / (10000.0 ** (np.arange(0, HEAD_DIM, 2, dtype=np.float32) / HEAD_DIM))
    sinusoid = np.arange(SEQ, dtype=np.float32)[:, None] * theta[None, :]
    sin = np.sin(sinusoid).astype(np.float32)
    cos = np.cos(sinusoid).astype(np.float32)
    scale = np.float32(1.0 / math.sqrt(HEAD_DIM))
    cq = (cos * scale).reshape(N_STILES, P, 64)
    sq = (sin * scale).reshape(N_STILES, P, 64)
    ck = cos.reshape(N_STILES, P, 64)
    sk = sin.reshape(N_STILES, P, 64)
    ident = np.eye(P, dtype=np.float32)

    perm = np.concatenate([np.arange(0, HEAD_DIM, 2), np.arange(1, HEAD_DIM, 2)])
    in_maps = []
    for c in range(N_CORES):
        cols_pk = np.concatenate([c * 256 + h * 128 + perm for h in range(HPC)])
        sl = slice(c * 256, (c + 1) * 256)
        in_maps.append({
            "hs": hs,
            "wq": np.ascontiguousarray(Wq.T[:, cols_pk]),
            "wk": np.ascontiguousarray(Wk.T[:, cols_pk]),
            "wv": np.ascontiguousarray(Wv.T[:, sl]),
            "wo": np.ascontiguousarray(Wo.T[sl, :]),
            "bq": np.ascontiguousarray(bq[cols_pk]).reshape(1, DLOC),
            "bk": np.ascontiguousarray(bk[cols_pk]).reshape(1, DLOC),
            "bv": np.ascontiguousarray(bv[sl]).reshape(1, DLOC),
            "bo": np.ascontiguousarray((bo / N_CORES).reshape(N_HTILES, P).T),
            "cq": cq, "sq": sq, "ck": ck, "sk": sk,
            "ident": ident,
        })
    return in_maps


def _maybe_enable_ldw_opt():
    """Experimental: the stock compile pipeline passes --enable-ldw-opt=false;
    flipping it on lets walrus pipeline LDWEIGHTS behind matmuls. Controlled
    by LDW_OPT=1 env; correctness is re-checked by the caller."""
    import os
    if os.environ.get("LDW_OPT", "0") != "1":
        return
    import concourse.bass_utils as bu
    if getattr(bu, "_ldw_patched", False):
        return
    orig = bu.bir_verify_and_optimise

    def patched(tmpdir, inp="bir.json", outp="file.neff", arch=None, **kw):
        import unittest.mock as um
        real_run = bu.run_command

        def run2(argv, **kwargs):
            argv = [a.replace("--enable-ldw-opt=false", "--enable-ldw-opt=true")
                    for a in argv]
            return real_run(argv, **kwargs)

        with um.patch.object(bu, "run_command", run2):
            return orig(tmpdir, inp, outp, arch, **kw)

    bu.bir_verify_and_optimise = patched
    # bass2jax binds its own reference
    import concourse.bass2jax as b2j
    if hasattr(b2j, "bir_verify_and_optimise"):
        b2j.bir_verify_and_optimise = patched
    bu._ldw_patched = True


def run(inputs, trace=False, mm="bf16"):
    _maybe_enable_ldw_opt()
    from concourse.bass_utils import run_bass_kernel_spmd
    key = mm
    if key not in _CACHE:
        _CACHE[key] = _build_program(mm)
    nc = _CACHE[key]
    in_maps = _host_inputs(inputs)
    res = run_bass_kernel_spmd(nc, in_maps, core_ids=list(range(N_CORES)),
                               trace=trace)
    acc = np.zeros((HID, SEQ), np.float64)
    for r in res.results:
        acc += r["pout"].astype(np.float64)
    out = acc.T.astype(np.float32).reshape(1, SEQ, HID)
    return out, res


def kernel(**inputs) -> np.ndarray:
    out, _ = run(inputs, trace=False)
    return out


# revision 29
# speedup vs baseline: 2.0655x; 1.0036x over previous
"""Trainium2 Bass kernel for MiniGPT4 CAM sparse attention.

Sharding: tensor-parallel over 16 heads -> 2 heads per core (8 cores).
Each core: q/k/v projections for its 2 heads (k/v only at the CAM-allowed
kv positions: [0,410) + [3072,4096)), RoPE, attention with the softmax
denominator folded into an augmented-V matmul (ones column), and a
row-sharded output projection producing a partial [2048, 4096] final^T.
Partials are summed on the host (gather/unshard step).

The three phases are interleaved into one pipeline: the CAM kv subtiles
are projected first, so attention for a query block starts as soon as its
q^T columns exist, and the output projection follows per query block.
Matmuls run in bf16 (full-rate PE); accumulation fp32 in PSUM.
"""

import math
import numpy as np

HID = 2048
HEADS = 16
HEAD_DIM = 128
SEQ = 4096
N_CORES = 8
HPC = HEADS // N_CORES          # heads per core = 2
DLOC = HPC * HEAD_DIM           # 256 local head dims
START = math.ceil(0.1 * SEQ)    # 410
RECENT = math.ceil(0.25 * SEQ)  # 1024
KV_HI = SEQ - RECENT            # 3072
NKV = START + RECENT            # 1434
SHIFT = 15.0                    # global exp shift (cancels in softmax)

P = 128
N_STILES = SEQ // P             # 32 seq subtiles
N_HTILES = HID // P             # 16 hidden tiles
QB = 512                        # q block (free dim of scores matmuls)
N_QB = SEQ // QB                # 8

# kv subtiles: (seq_subtile_index, rows_used) covering [0,410) + [3072,4096)
KV_STILES = [(0, 128), (1, 128), (2, 128), (3, 26)] + [(24 + i, 128) for i in range(8)]
N_KVT = len(KV_STILES)          # 12
KV_COL = np.cumsum([0] + [r for _, r in KV_STILES]).tolist()
NKV_PAD = 1440

_CACHE = {}


def _build_program(mm="bf16"):
    import concourse.bass as bass
    import concourse.bacc as bacc
    import concourse.mybir as mybir
    from concourse.tile import TileContext
    from contextlib import ExitStack

    fp32 = mybir.dt.float32
    bf16 = mybir.dt.bfloat16
    AF = mybir.ActivationFunctionType
    ALU = mybir.AluOpType

    nc = bacc.Bacc()

    hs_d = nc.declare_dram_parameter("hs", [SEQ, HID], fp32, isOutput=False)
    wq_d = nc.declare_dram_parameter("wq", [HID, DLOC], fp32, isOutput=False)
    wk_d = nc.declare_dram_parameter("wk", [HID, DLOC], fp32, isOutput=False)
    wv_d = nc.declare_dram_parameter("wv", [HID, DLOC], fp32, isOutput=False)
    wo_d = nc.declare_dram_parameter("wo", [DLOC, HID], fp32, isOutput=False)
    bq_d = nc.declare_dram_parameter("bq", [1, DLOC], fp32, isOutput=False)
    bk_d = nc.declare_dram_parameter("bk", [1, DLOC], fp32, isOutput=False)
    bv_d = nc.declare_dram_parameter("bv", [1, DLOC], fp32, isOutput=False)
    bo_d = nc.declare_dram_parameter("bo", [P, N_HTILES], fp32, isOutput=False)
    cq_d = nc.declare_dram_parameter("cq", [N_STILES, P, 64], fp32, isOutput=False)
    sq_d = nc.declare_dram_parameter("sq", [N_STILES, P, 64], fp32, isOutput=False)
    ck_d = nc.declare_dram_parameter("ck", [N_STILES, P, 64], fp32, isOutput=False)
    sk_d = nc.declare_dram_parameter("sk", [N_STILES, P, 64], fp32, isOutput=False)
    id_d = nc.declare_dram_parameter("ident", [P, P], fp32, isOutput=False)
    out_d = nc.declare_dram_parameter("pout", [HID, SEQ], fp32, isOutput=True)

    with TileContext(nc) as tc, ExitStack() as top:
        const = top.enter_context(tc.tile_pool(name="const", bufs=1))
        persist = top.enter_context(tc.tile_pool(name="persist", bufs=1))
        work = top.enter_context(tc.tile_pool(name="work", bufs=1))
        psum = top.enter_context(tc.tile_pool(name="psum", bufs=1, space="PSUM"))

        # ---- constants (bf16 matmul operands via SWDGE cast DMA) ----
        # fused [wq | wk] per h-tile so kv subtiles project q and k together
        wqk_sb = const.tile([P, N_HTILES, 2 * DLOC], bf16)
        nc.gpsimd.dma_start(
            wqk_sb[:].rearrange("p t (x d) -> p t x d", x=2)[:, :, 0, :],
            wq_d[:].rearrange("(t p) d -> p t d", p=P))
        nc.gpsimd.dma_start(
            wqk_sb[:].rearrange("p t (x d) -> p t x d", x=2)[:, :, 1, :],
            wk_d[:].rearrange("(t p) d -> p t d", p=P))
        wv_sb = const.tile([P, N_HTILES, DLOC], bf16)
        nc.gpsimd.dma_start(wv_sb[:], wv_d[:].rearrange("(t p) d -> p t d", p=P))
        wo_sb = const.tile([P, HPC, HID], bf16)
        nc.gpsimd.dma_start(wo_sb[:], wo_d[:].rearrange("(t p) e -> p t e", p=P))

        bqk_sb = const.tile([1, 2 * DLOC], bf16)
        nc.gpsimd.dma_start(bqk_sb[:, 0:DLOC], bq_d[:])
        nc.gpsimd.dma_start(bqk_sb[:, DLOC:2 * DLOC], bk_d[:])
        bv_sb = const.tile([1, DLOC], bf16)
        nc.gpsimd.dma_start(bv_sb[:], bv_d[:])
        boc_sb = const.tile([P, N_HTILES], fp32)
        nc.sync.dma_start(boc_sb[:], bo_d[:])
        ident_sb = const.tile([P, P], bf16)
        nc.gpsimd.dma_start(ident_sb[:], id_d[:])
        cq_sb = const.tile([P, N_STILES, 64], fp32)
        sq_sb = const.tile([P, N_STILES, 64], fp32)
        nc.sync.dma_start(cq_sb[:], cq_d[:].rearrange("t p d -> p t d"))
        nc.sync.dma_start(sq_sb[:], sq_d[:].rearrange("t p d -> p t d"))
        ones_sb = const.tile([1, QB], bf16)
        nc.gpsimd.memset(ones_sb[:], 1.0)
        onescol_sb = const.tile([P, 1], bf16)
        nc.gpsimd.memset(onescol_sb[:], 1.0)
        shift_sb = const.tile([P, 1], fp32)
        nc.gpsimd.memset(shift_sb[:], -SHIFT)

        # ---- persistent activations ----
        qT_sb = persist.tile([P, HPC, SEQ], bf16)
        kT_sb = persist.tile([P, HPC, NKV_PAD], bf16)
        vA_sb = persist.tile([P, HPC, N_KVT, P], bf16)
        oT_sb = persist.tile([P, HPC, SEQ], bf16)

        kv_of_stile = {st: (i, rows) for i, (st, rows) in enumerate(KV_STILES)}

        # ---------------- phase A worker: one seq subtile ----------------
        def do_subtile(st):
            is_kv = st in kv_of_stile
            hs_t = work.tile([P, HID], bf16, tag="hs", bufs=3, name=f"hs{st}")
            nc.gpsimd.dma_start(hs_t[:], hs_d[st * P:(st + 1) * P, :])

            hsT = work.tile([P, N_HTILES, P], bf16, tag="hsT", bufs=3,
                            name=f"hsT{st}")
            for g in range(N_HTILES // 4):
                ps = psum.tile([P, 4 * P], bf16, tag="tp", bufs=1, name=f"tp{st}_{g}")
                for j in range(4):
                    ht = g * 4 + j
                    nc.tensor.transpose(ps[:, j * P:(j + 1) * P],
                                        hs_t[:, ht * P:(ht + 1) * P], ident_sb[:])
                nc.scalar.copy(
                    hsT[:, g * 4:(g + 1) * 4, :].rearrange("p a b -> p (a b)"),
                    ps[:])

            # q (and on kv subtiles also k) projection, fused rhs [wq|wk]
            nw = 2 * DLOC if is_kv else DLOC
            qkp = psum.tile([P, 2 * DLOC], fp32, tag="qp", bufs=2, name=f"qk{st}")
            nc.tensor.matmul(qkp[:, 0:nw], ones_sb[:, 0:P], bqk_sb[:, 0:nw],
                             start=True, stop=False)
            for ht in range(N_HTILES):
                nc.tensor.matmul(qkp[:, 0:nw], hsT[:, ht, :], wqk_sb[:, ht, 0:nw],
                                 start=False, stop=(ht == N_HTILES - 1))

            def rope(dst, src, cos_ap, sin_ap, rows=P):
                sr = src.rearrange("p (h x d) -> p h x d", h=HPC, x=2)
                dr = dst.rearrange("p (h x d) -> p h x d", h=HPC, x=2)
                x1, x2 = sr[:, :, 0, :], sr[:, :, 1, :]
                o1, o2 = dr[:, :, 0, :], dr[:, :, 1, :]
                cb = cos_ap.rearrange("p (o d) -> p o d", o=1).broadcast_to([rows, HPC, 64])
                sbb = sin_ap.rearrange("p (o d) -> p o d", o=1).broadcast_to([rows, HPC, 64])
                tmp = work.tile([P, P], fp32, tag="rtmp", bufs=3, name=f"rt{st}")
                tr = tmp[0:rows, :].rearrange("p (h d) -> p h d", h=HPC)
                nc.vector.tensor_tensor(o1, x1, cb, ALU.mult)
                nc.vector.tensor_tensor(tr, x2, sbb, ALU.mult)
                nc.vector.tensor_tensor(o1, o1, tr, ALU.subtract)
                nc.vector.tensor_tensor(o2, x1, sbb, ALU.mult)
                nc.vector.tensor_tensor(tr, x2, cb, ALU.mult)
                nc.vector.tensor_tensor(o2, o2, tr, ALU.add)

            qr = work.tile([P, DLOC], bf16, tag="qr", bufs=3, name=f"qr{st}")
            rope(qr[:], qkp[:, 0:DLOC], cq_sb[:, st, :], sq_sb[:, st, :])
            # transpose roped q into qT (both heads -> one psum, one copy)
            pq = psum.tile([P, 2 * P], bf16, tag="tq", bufs=1, name=f"pq{st}")
            for h in range(HPC):
                nc.tensor.transpose(pq[:, h * P:(h + 1) * P],
                                    qr[:, h * P:(h + 1) * P], ident_sb[:])
            nc.vector.tensor_copy(
                qT_sb[:, :, st * P:(st + 1) * P],
                pq[:].rearrange("p (a b) -> p a b", a=HPC))

            if is_kv:
                ti, rows = kv_of_stile[st]
                col = KV_COL[ti]
                ck_t = work.tile([P, 64], fp32, tag="ck", bufs=2, name=f"ck{st}")
                sk_t = work.tile([P, 64], fp32, tag="sk", bufs=2, name=f"sk{st}")
                nc.sync.dma_start(ck_t[:rows, :], ck_d[st, 0:rows, :])
                nc.sync.dma_start(sk_t[:rows, :], sk_d[st, 0:rows, :])

                kr = work.tile([P, DLOC], bf16, tag="qr", bufs=3, name=f"kr{st}")
                rope(kr[0:rows, :], qkp[0:rows, DLOC:2 * DLOC],
                     ck_t[0:rows, :], sk_t[0:rows, :], rows=rows)
                for h in range(HPC):
                    pk = psum.tile([P, P], bf16, tag="tq", bufs=1,
                                   name=f"pk{st}_{h}")
                    nc.tensor.transpose(pk[:, 0:rows],
                                        kr[0:rows, h * P:(h + 1) * P],
                                        ident_sb[0:rows, 0:rows])
                    nc.vector.tensor_copy(kT_sb[:, h, col:col + rows],
                                          pk[:, 0:rows])

                vp = psum.tile([P, 2 * DLOC], fp32, tag="qp", bufs=2, name=f"v{st}")
                nc.tensor.matmul(vp[0:rows, 0:DLOC], ones_sb[:, 0:rows], bv_sb[:],
                                 start=True, stop=False)
                for ht in range(N_HTILES):
                    nc.tensor.matmul(vp[0:rows, 0:DLOC], hsT[:, ht, 0:rows],
                                     wv_sb[:, ht, :],
                                     start=False, stop=(ht == N_HTILES - 1))
                for h in range(HPC):
                    nc.vector.tensor_copy(vA_sb[0:rows, h, ti, 0:P],
                                          vp[0:rows, h * P:(h + 1) * P])

        # ---------------- phase B workers: attention for one q block ------
        ex_tiles = {}

        def do_scores(qb):
            for h in range(HPC):
                ex = work.tile([P, N_KVT, QB], bf16, tag="exp", bufs=4,
                               name=f"ex{qb}_{h}")
                ex_tiles[(qb, h)] = ex
                for ti, (st, rows) in enumerate(KV_STILES):
                    col = KV_COL[ti]
                    sp = psum.tile([P, QB], fp32, tag="sc", bufs=2,
                                   name=f"sc{qb}_{h}_{ti}")
                    nc.tensor.matmul(sp[0:rows, :],
                                     kT_sb[:, h, col:col + rows],
                                     qT_sb[:, h, qb * QB:(qb + 1) * QB],
                                     start=True, stop=True)
                    nc.scalar.activation(ex[0:rows, ti, :], sp[0:rows, :],
                                         AF.Exp, bias=shift_sb[0:rows, :],
                                         scale=1.0)

        # kv tiles with full 128 rows, for the denominator pair-tree
        FULL_TIS = [ti for ti, (st, r) in enumerate(KV_STILES) if r == P]
        SHORT_TI = [ti for ti, (st, r) in enumerate(KV_STILES) if r != P][0]

        def do_attnv(qb):
            for h in range(HPC):
                ex = ex_tiles.pop((qb, h))
                # O^T accumulation [128d, 512q] with N=512 matmuls
                op = psum.tile([P, QB], fp32, tag="ov", bufs=2,
                               name=f"ov{qb}_{h}")
                dn = psum.tile([1, QB], fp32, tag="ov", bufs=2,
                               name=f"dn{qb}_{h}")
                for ti, (st, rows) in enumerate(KV_STILES):
                    nc.tensor.matmul(op[:],
                                     vA_sb[0:rows, h, ti, :],
                                     ex[0:rows, ti, :],
                                     start=(ti == 0), stop=(ti == N_KVT - 1))
                    nc.tensor.matmul(dn[:],
                                     onescol_sb[0:rows, :],
                                     ex[0:rows, ti, :],
                                     start=(ti == 0), stop=(ti == N_KVT - 1))
                recip = work.tile([1, QB], fp32, tag="recip", bufs=3,
                                  name=f"rc{qb}_{h}")
                nc.vector.reciprocal(recip[:], dn[:])
                rb = work.tile([P, QB], fp32, tag="rb", bufs=3,
                               name=f"rb{qb}_{h}")
                nc.gpsimd.partition_broadcast(rb[:], recip[:])
                nc.vector.tensor_tensor(oT_sb[:, h, qb * QB:(qb + 1) * QB],
                                        op[:], rb[:], ALU.mult)

        # ---------------- phase C worker: out-proj for one q block --------
        def do_oproj(qb):
            for et in range(N_HTILES):
                fp = psum.tile([P, QB], fp32, tag="qp", bufs=2,
                               name=f"fp{qb}_{et}")
                for h in range(HPC):
                    nc.tensor.matmul(fp[:],
                                     wo_sb[:, h, et * P:(et + 1) * P],
                                     oT_sb[:, h, qb * QB:(qb + 1) * QB],
                                     start=(h == 0), stop=(h == HPC - 1))
                stg = work.tile([P, QB], fp32, tag="stage", bufs=4,
                                name=f"st{qb}_{et}")
                # copy PSUM->SBUF with the (per-partition) bo bias folded in
                nc.vector.tensor_scalar_add(stg[:], fp[:],
                                            boc_sb[:, et:et + 1])
                nc.sync.dma_start(
                    out_d[et * P:(et + 1) * P, qb * QB:(qb + 1) * QB], stg[:])

        # ---------------- interleaved pipeline ----------------
        # kv subtiles first; then per q block: scores -> (weave in remaining
        # subtiles so the PE never head-of-line-blocks on ACT's exp) ->
        # attention@V -> more subtiles -> out-projection.
        from collections import deque
        for st in [st for st, _ in KV_STILES]:
            do_subtile(st)
        pending = deque(range(4, 24))
        emitted = set(st for st, _ in KV_STILES)

        def emit_a(n):
            for _ in range(n):
                if pending:
                    st = pending.popleft()
                    do_subtile(st)
                    emitted.add(st)

        for qb in (0, 6, 7, 1, 2, 3, 4, 5):
            while not all(s in emitted for s in range(qb * 4, qb * 4 + 4)):
                emit_a(1)
            do_scores(qb)
            emit_a(2)
            do_attnv(qb)
            emit_a(1)
            do_oproj(qb)

    nc.finalize()
    return nc


def _host_inputs(inputs):
    hs = np.ascontiguousarray(np.asarray(inputs["hidden_states"], np.float32).reshape(SEQ, HID))
    Wq = np.asarray(inputs["Wq"], np.float32)
    Wk = np.asarray(inputs["Wk"], np.float32)
    Wv = np.asarray(inputs["Wv"], np.float32)
    Wo = np.asarray(inputs["Wo"], np.float32)
    bq = np.asarray(inputs["bq"], np.float32)
    bk = np.asarray(inputs["bk"], np.float32)
    bv = np.asarray(inputs["bv"], np.float32)
    bo = np.asarray(inputs["bo"], np.float32)

    theta = 1.0 / (10000.0 ** (np.arange(0, HEAD_DIM, 2, dtype=np.float32) / HEAD_DIM))
    sinusoid = np.arange(SEQ, dtype=np.float32)[:, None] * theta[None, :]
    sin = np.sin(sinusoid).astype(np.float32)
    cos = np.cos(sinusoid).astype(np.float32)
    scale = np.float32(1.0 / math.sqrt(HEAD_DIM))
    cq = (cos * scale).reshape(N_STILES, P, 64)
    sq = (sin * scale).reshape(N_STILES, P, 64)
    ck = cos.reshape(N_STILES, P, 64)
    sk = sin.reshape(N_STILES, P, 64)
    ident = np.eye(P, dtype=np.float32)

    perm = np.concatenate([np.arange(0, HEAD_DIM, 2), np.arange(1, HEAD_DIM, 2)])
    in_maps = []
    for c in range(N_CORES):
        cols_pk = np.concatenate([c * 256 + h * 128 + perm for h in range(HPC)])
        sl = slice(c * 256, (c + 1) * 256)
        in_maps.append({
            "hs": hs,
            "wq": np.ascontiguousarray(Wq.T[:, cols_pk]),
            "wk": np.ascontiguousarray(Wk.T[:, cols_pk]),
            "wv": np.ascontiguousarray(Wv.T[:, sl]),
            "wo": np.ascontiguousarray(Wo.T[sl, :]),
            "bq": np.ascontiguousarray(bq[cols_pk]).reshape(1, DLOC),
            "bk": np.ascontiguousarray(bk[cols_pk]).reshape(1, DLOC),
            "bv": np.ascontiguousarray(bv[sl]).reshape(1, DLOC),
            "bo": np.ascontiguousarray((bo / N_CORES).reshape(N_HTILES, P).T),
            "cq": cq, "sq": sq, "ck": ck, "sk": sk,
            "ident": ident,
        })
    return in_maps


def _maybe_enable_ldw_opt():
    """Experimental: the stock compile pipeline passes --enable-ldw-opt=false;
    flipping it on lets walrus pipeline LDWEIGHTS behind matmuls. Controlled
    by LDW_OPT=1 env; correctness is re-checked by the caller."""
    import os
    if os.environ.get("LDW_OPT", "0") != "1":
        return
    import concourse.bass_utils as bu
    if getattr(bu, "_ldw_patched", False):
        return
    orig = bu.bir_verify_and_optimise

    def patched(tmpdir, inp="bir.json", outp="file.neff", arch=None, **kw):
        import unittest.mock as um
        real_run = bu.run_command

        def run2(argv, **kwargs):
            argv = [a.replace("--enable-ldw-opt=false", "--enable-ldw-opt=true")
                    for a in argv]
            return real_run(argv, **kwargs)

        with um.patch.object(bu, "run_command", run2):
            return orig(tmpdir, inp, outp, arch, **kw)

    bu.bir_verify_and_optimise = patched
    # bass2jax binds its own reference
    import concourse.bass2jax as b2j
    if hasattr(b2j, "bir_verify_and_optimise"):
        b2j.bir_verify_and_optimise = patched
    bu._ldw_patched = True


def run(inputs, trace=False, mm="bf16"):
    _maybe_enable_ldw_opt()
    from concourse.bass_utils import run_bass_kernel_spmd
    key = mm
    if key not in _CACHE:
        _CACHE[key] = _build_program(mm)
    nc = _CACHE[key]
    in_maps = _host_inputs(inputs)
    res = run_bass_kernel_spmd(nc, in_maps, core_ids=list(range(N_CORES)),
                               trace=trace)
    acc = np.zeros((HID, SEQ), np.float64)
    for r in res.results:
        acc += r["pout"].astype(np.float64)
    out = acc.T.astype(np.float32).reshape(1, SEQ, HID)
    return out, res


def kernel(**inputs) -> np.ndarray:
    out, _ = run(inputs, trace=False)
    return out


# revision 30
# speedup vs baseline: 2.1934x; 1.0619x over previous
"""Trainium2 Bass kernel for MiniGPT4 CAM sparse attention.

Sharding: tensor-parallel over 16 heads -> 2 heads per core (8 cores).
Each core: q/k/v projections for its 2 heads (k/v only at the CAM-allowed
kv positions: [0,410) + [3072,4096)), RoPE, attention with the softmax
denominator folded into an augmented-V matmul (ones column), and a
row-sharded output projection producing a partial [2048, 4096] final^T.
Partials are summed on the host (gather/unshard step).

The three phases are interleaved into one pipeline: the CAM kv subtiles
are projected first, so attention for a query block starts as soon as its
q^T columns exist, and the output projection follows per query block.
Matmuls run in bf16 (full-rate PE); accumulation fp32 in PSUM.
"""

import math
import numpy as np

HID = 2048
HEADS = 16
HEAD_DIM = 128
SEQ = 4096
N_CORES = 8
HPC = HEADS // N_CORES          # heads per core = 2
DLOC = HPC * HEAD_DIM           # 256 local head dims
START = math.ceil(0.1 * SEQ)    # 410
RECENT = math.ceil(0.25 * SEQ)  # 1024
KV_HI = SEQ - RECENT            # 3072
NKV = START + RECENT            # 1434
SHIFT = 15.0                    # global exp shift (cancels in softmax)

P = 128
N_STILES = SEQ // P             # 32 seq subtiles
N_HTILES = HID // P             # 16 hidden tiles
QB = 512                        # q block (free dim of scores matmuls)
N_QB = SEQ // QB                # 8

# kv subtiles: (seq_subtile_index, rows_used) covering [0,410) + [3072,4096)
KV_STILES = [(0, 128), (1, 128), (2, 128), (3, 26)] + [(24 + i, 128) for i in range(8)]
N_KVT = len(KV_STILES)          # 12
KV_COL = np.cumsum([0] + [r for _, r in KV_STILES]).tolist()
NKV_PAD = 1440

_CACHE = {}


def _build_program(mm="bf16"):
    import concourse.bass as bass
    import concourse.bacc as bacc
    import concourse.mybir as mybir
    from concourse.tile import TileContext
    from contextlib import ExitStack

    fp32 = mybir.dt.float32
    bf16 = mybir.dt.bfloat16
    AF = mybir.ActivationFunctionType
    ALU = mybir.AluOpType

    nc = bacc.Bacc()

    hs_d = nc.declare_dram_parameter("hs", [SEQ, HID], fp32, isOutput=False)
    wq_d = nc.declare_dram_parameter("wq", [HID, DLOC], fp32, isOutput=False)
    wk_d = nc.declare_dram_parameter("wk", [HID, DLOC], fp32, isOutput=False)
    wv_d = nc.declare_dram_parameter("wv", [HID, DLOC], fp32, isOutput=False)
    wo_d = nc.declare_dram_parameter("wo", [DLOC, HID], fp32, isOutput=False)
    bq_d = nc.declare_dram_parameter("bq", [1, DLOC], fp32, isOutput=False)
    bk_d = nc.declare_dram_parameter("bk", [1, DLOC], fp32, isOutput=False)
    bv_d = nc.declare_dram_parameter("bv", [1, DLOC], fp32, isOutput=False)
    bo_d = nc.declare_dram_parameter("bo", [P, N_HTILES], fp32, isOutput=False)
    cq_d = nc.declare_dram_parameter("cq", [N_STILES, P, 64], fp32, isOutput=False)
    sq_d = nc.declare_dram_parameter("sq", [N_STILES, P, 64], fp32, isOutput=False)
    ck_d = nc.declare_dram_parameter("ck", [N_STILES, P, 64], fp32, isOutput=False)
    sk_d = nc.declare_dram_parameter("sk", [N_STILES, P, 64], fp32, isOutput=False)
    id_d = nc.declare_dram_parameter("ident", [P, P], fp32, isOutput=False)
    out_d = nc.declare_dram_parameter("pout", [HID, SEQ], fp32, isOutput=True)

    with TileContext(nc) as tc, ExitStack() as top:
        const = top.enter_context(tc.tile_pool(name="const", bufs=1))
        persist = top.enter_context(tc.tile_pool(name="persist", bufs=1))
        work = top.enter_context(tc.tile_pool(name="work", bufs=1))
        psum = top.enter_context(tc.tile_pool(name="psum", bufs=1, space="PSUM"))

        # ---- constants (bf16 matmul operands via SWDGE cast DMA) ----
        # fused [wq | wk] per h-tile so kv subtiles project q and k together
        wqk_sb = const.tile([P, N_HTILES, 2 * DLOC], bf16)
        nc.gpsimd.dma_start(
            wqk_sb[:].rearrange("p t (x d) -> p t x d", x=2)[:, :, 0, :],
            wq_d[:].rearrange("(t p) d -> p t d", p=P))
        nc.gpsimd.dma_start(
            wqk_sb[:].rearrange("p t (x d) -> p t x d", x=2)[:, :, 1, :],
            wk_d[:].rearrange("(t p) d -> p t d", p=P))
        wv_sb = const.tile([P, N_HTILES, DLOC], bf16)
        nc.gpsimd.dma_start(wv_sb[:], wv_d[:].rearrange("(t p) d -> p t d", p=P))
        wo_sb = const.tile([P, HPC, HID], bf16)
        nc.gpsimd.dma_start(wo_sb[:], wo_d[:].rearrange("(t p) e -> p t e", p=P))

        bqk_sb = const.tile([1, 2 * DLOC], bf16)
        nc.gpsimd.dma_start(bqk_sb[:, 0:DLOC], bq_d[:])
        nc.gpsimd.dma_start(bqk_sb[:, DLOC:2 * DLOC], bk_d[:])
        bv_sb = const.tile([1, DLOC], bf16)
        nc.gpsimd.dma_start(bv_sb[:], bv_d[:])
        boc_sb = const.tile([P, N_HTILES], fp32)
        nc.sync.dma_start(boc_sb[:], bo_d[:])
        ident_sb = const.tile([P, P], bf16)
        nc.gpsimd.dma_start(ident_sb[:], id_d[:])
        cq_sb = const.tile([P, N_STILES, 64], fp32)
        sq_sb = const.tile([P, N_STILES, 64], fp32)
        nc.sync.dma_start(cq_sb[:], cq_d[:].rearrange("t p d -> p t d"))
        nc.sync.dma_start(sq_sb[:], sq_d[:].rearrange("t p d -> p t d"))
        ones_sb = const.tile([1, QB], bf16)
        nc.gpsimd.memset(ones_sb[:], 1.0)
        onescol_sb = const.tile([P, 1], bf16)
        nc.gpsimd.memset(onescol_sb[:], 1.0)
        shift_sb = const.tile([P, 1], fp32)
        nc.gpsimd.memset(shift_sb[:], -SHIFT)

        # ---- persistent activations ----
        qT_sb = persist.tile([P, HPC, SEQ], bf16)
        kT_sb = persist.tile([P, HPC, NKV_PAD], bf16)
        vA_sb = persist.tile([P, HPC, N_KVT, P], bf16)
        oT_sb = persist.tile([P, HPC, SEQ], bf16)

        kv_of_stile = {st: (i, rows) for i, (st, rows) in enumerate(KV_STILES)}

        # ---------------- phase A worker: one seq subtile ----------------
        def do_subtile(st):
            is_kv = st in kv_of_stile
            hs_t = work.tile([P, HID], bf16, tag="hs", bufs=3, name=f"hs{st}")
            nc.gpsimd.dma_start(hs_t[:], hs_d[st * P:(st + 1) * P, :])

            hsT = work.tile([P, N_HTILES, P], bf16, tag="hsT", bufs=3,
                            name=f"hsT{st}")
            for g in range(N_HTILES // 4):
                ps = psum.tile([P, 4 * P], bf16, tag="tp", bufs=1, name=f"tp{st}_{g}")
                for j in range(4):
                    ht = g * 4 + j
                    nc.tensor.transpose(ps[:, j * P:(j + 1) * P],
                                        hs_t[:, ht * P:(ht + 1) * P], ident_sb[:])
                nc.scalar.copy(
                    hsT[:, g * 4:(g + 1) * 4, :].rearrange("p a b -> p (a b)"),
                    ps[:])

            # q (and on kv subtiles also k) projection, fused rhs [wq|wk]
            nw = 2 * DLOC if is_kv else DLOC
            qkp = psum.tile([P, 2 * DLOC], fp32, tag="qp", bufs=2, name=f"qk{st}")
            nc.tensor.matmul(qkp[:, 0:nw], ones_sb[:, 0:P], bqk_sb[:, 0:nw],
                             start=True, stop=False)
            for ht in range(N_HTILES):
                nc.tensor.matmul(qkp[:, 0:nw], hsT[:, ht, :], wqk_sb[:, ht, 0:nw],
                                 start=False, stop=(ht == N_HTILES - 1))

            def rope(dst, src, cos_ap, sin_ap, rows=P):
                sr = src.rearrange("p (h x d) -> p h x d", h=HPC, x=2)
                dr = dst.rearrange("p (h x d) -> p h x d", h=HPC, x=2)
                x1, x2 = sr[:, :, 0, :], sr[:, :, 1, :]
                o1, o2 = dr[:, :, 0, :], dr[:, :, 1, :]
                cb = cos_ap.rearrange("p (o d) -> p o d", o=1).broadcast_to([rows, HPC, 64])
                sbb = sin_ap.rearrange("p (o d) -> p o d", o=1).broadcast_to([rows, HPC, 64])
                tmp = work.tile([P, P], fp32, tag="rtmp", bufs=3, name=f"rt{st}")
                tr = tmp[0:rows, :].rearrange("p (h d) -> p h d", h=HPC)
                nc.vector.tensor_tensor(o1, x1, cb, ALU.mult)
                nc.vector.tensor_tensor(tr, x2, sbb, ALU.mult)
                nc.vector.tensor_tensor(o1, o1, tr, ALU.subtract)
                nc.vector.tensor_tensor(o2, x1, sbb, ALU.mult)
                nc.vector.tensor_tensor(tr, x2, cb, ALU.mult)
                nc.vector.tensor_tensor(o2, o2, tr, ALU.add)

            qr = work.tile([P, DLOC], bf16, tag="qr", bufs=3, name=f"qr{st}")
            rope(qr[:], qkp[:, 0:DLOC], cq_sb[:, st, :], sq_sb[:, st, :])
            # transpose roped q into qT (both heads -> one psum, one copy)
            pq = psum.tile([P, 2 * P], bf16, tag="tq", bufs=1, name=f"pq{st}")
            for h in range(HPC):
                nc.tensor.transpose(pq[:, h * P:(h + 1) * P],
                                    qr[:, h * P:(h + 1) * P], ident_sb[:])
            nc.vector.tensor_copy(
                qT_sb[:, :, st * P:(st + 1) * P],
                pq[:].rearrange("p (a b) -> p a b", a=HPC))

            if is_kv:
                ti, rows = kv_of_stile[st]
                col = KV_COL[ti]
                ck_t = work.tile([P, 64], fp32, tag="ck", bufs=2, name=f"ck{st}")
                sk_t = work.tile([P, 64], fp32, tag="sk", bufs=2, name=f"sk{st}")
                nc.sync.dma_start(ck_t[:rows, :], ck_d[st, 0:rows, :])
                nc.sync.dma_start(sk_t[:rows, :], sk_d[st, 0:rows, :])

                kr = work.tile([P, DLOC], bf16, tag="qr", bufs=3, name=f"kr{st}")
                rope(kr[0:rows, :], qkp[0:rows, DLOC:2 * DLOC],
                     ck_t[0:rows, :], sk_t[0:rows, :], rows=rows)
                for h in range(HPC):
                    pk = psum.tile([P, P], bf16, tag="tq", bufs=1,
                                   name=f"pk{st}_{h}")
                    nc.tensor.transpose(pk[:, 0:rows],
                                        kr[0:rows, h * P:(h + 1) * P],
                                        ident_sb[0:rows, 0:rows])
                    nc.vector.tensor_copy(kT_sb[:, h, col:col + rows],
                                          pk[:, 0:rows])

                vp = psum.tile([P, 2 * DLOC], fp32, tag="qp", bufs=2, name=f"v{st}")
                nc.tensor.matmul(vp[0:rows, 0:DLOC], ones_sb[:, 0:rows], bv_sb[:],
                                 start=True, stop=False)
                for ht in range(N_HTILES):
                    nc.tensor.matmul(vp[0:rows, 0:DLOC], hsT[:, ht, 0:rows],
                                     wv_sb[:, ht, :],
                                     start=False, stop=(ht == N_HTILES - 1))
                for h in range(HPC):
                    nc.vector.tensor_copy(vA_sb[0:rows, h, ti, 0:P],
                                          vp[0:rows, h * P:(h + 1) * P])

        # ---------------- phase B workers: attention for one q block ------
        ex_tiles = {}

        def do_scores(qb):
            for h in range(HPC):
                ex = work.tile([P, N_KVT, QB], bf16, tag="exp", bufs=4,
                               name=f"ex{qb}_{h}")
                ex_tiles[(qb, h)] = ex
                for ti, (st, rows) in enumerate(KV_STILES):
                    col = KV_COL[ti]
                    sp = psum.tile([P, QB], fp32, tag="sc", bufs=2,
                                   name=f"sc{qb}_{h}_{ti}")
                    nc.tensor.matmul(sp[0:rows, :],
                                     kT_sb[:, h, col:col + rows],
                                     qT_sb[:, h, qb * QB:(qb + 1) * QB],
                                     start=True, stop=True)
                    nc.scalar.activation(ex[0:rows, ti, :], sp[0:rows, :],
                                         AF.Exp, bias=shift_sb[0:rows, :],
                                         scale=1.0)

        # kv tiles with full 128 rows, for the denominator pair-tree
        FULL_TIS = [ti for ti, (st, r) in enumerate(KV_STILES) if r == P]
        SHORT_TI = [ti for ti, (st, r) in enumerate(KV_STILES) if r != P][0]

        def do_attnv(qb):
            for h in range(HPC):
                ex = ex_tiles.pop((qb, h))
                # O^T accumulation [128d, 512q] with N=512 matmuls
                op = psum.tile([P, QB], fp32, tag="ov", bufs=2,
                               name=f"ov{qb}_{h}")
                dn = psum.tile([1, QB], fp32, tag="tq", bufs=1,
                               name=f"dn{qb}_{h}")
                for ti, (st, rows) in enumerate(KV_STILES):
                    nc.tensor.matmul(op[:],
                                     vA_sb[0:rows, h, ti, :],
                                     ex[0:rows, ti, :],
                                     start=(ti == 0), stop=(ti == N_KVT - 1))
                    nc.tensor.matmul(dn[:],
                                     onescol_sb[0:rows, :],
                                     ex[0:rows, ti, :],
                                     start=(ti == 0), stop=(ti == N_KVT - 1))
                recip = work.tile([1, QB], fp32, tag="recip", bufs=3,
                                  name=f"rc{qb}_{h}")
                nc.vector.reciprocal(recip[:], dn[:])
                rb = work.tile([P, QB], fp32, tag="rb", bufs=3,
                               name=f"rb{qb}_{h}")
                nc.gpsimd.partition_broadcast(rb[:], recip[:])
                nc.vector.tensor_tensor(oT_sb[:, h, qb * QB:(qb + 1) * QB],
                                        op[:], rb[:], ALU.mult)

        # ---------------- phase C worker: out-proj for one q block --------
        def do_oproj(qb):
            for et in range(N_HTILES):
                fp = psum.tile([P, QB], fp32, tag="qp", bufs=2,
                               name=f"fp{qb}_{et}")
                for h in range(HPC):
                    nc.tensor.matmul(fp[:],
                                     wo_sb[:, h, et * P:(et + 1) * P],
                                     oT_sb[:, h, qb * QB:(qb + 1) * QB],
                                     start=(h == 0), stop=(h == HPC - 1))
                stg = work.tile([P, QB], fp32, tag="stage", bufs=4,
                                name=f"st{qb}_{et}")
                # copy PSUM->SBUF with the (per-partition) bo bias folded in
                nc.vector.tensor_scalar_add(stg[:], fp[:],
                                            boc_sb[:, et:et + 1])
                nc.sync.dma_start(
                    out_d[et * P:(et + 1) * P, qb * QB:(qb + 1) * QB], stg[:])

        # ---------------- interleaved pipeline ----------------
        # kv subtiles first; then per q block: scores -> (weave in remaining
        # subtiles so the PE never head-of-line-blocks on ACT's exp) ->
        # attention@V -> more subtiles -> out-projection.
        from collections import deque
        for st in [st for st, _ in KV_STILES]:
            do_subtile(st)
        pending = deque(range(4, 24))
        emitted = set(st for st, _ in KV_STILES)

        def emit_a(n):
            for _ in range(n):
                if pending:
                    st = pending.popleft()
                    do_subtile(st)
                    emitted.add(st)

        for qb in (0, 6, 7, 1, 2, 3, 4, 5):
            while not all(s in emitted for s in range(qb * 4, qb * 4 + 4)):
                emit_a(1)
            do_scores(qb)
            emit_a(2)
            do_attnv(qb)
            emit_a(1)
            do_oproj(qb)

    nc.finalize()
    return nc


def _host_inputs(inputs):
    hs = np.ascontiguousarray(np.asarray(inputs["hidden_states"], np.float32).reshape(SEQ, HID))
    Wq = np.asarray(inputs["Wq"], np.float32)
    Wk = np.asarray(inputs["Wk"], np.float32)
    Wv = np.asarray(inputs["Wv"], np.float32)
    Wo = np.asarray(inputs["Wo"], np.float32)
    bq = np.asarray(inputs["bq"], np.float32)
    bk = np.asarray(inputs["bk"], np.float32)
    bv = np.asarray(inputs["bv"], np.float32)
    bo = np.asarray(inputs["bo"], np.float32)

    theta = 1.0 / (10000.0 ** (np.arange(0, HEAD_DIM, 2, dtype=np.float32) / HEAD_DIM))
    sinusoid = np.arange(SEQ, dtype=np.float32)[:, None] * theta[None, :]
    sin = np.sin(sinusoid).astype(np.float32)
    cos = np.cos(sinusoid).astype(np.float32)
    scale = np.float32(1.0 / math.sqrt(HEAD_DIM))
    cq = (cos * scale).reshape(N_STILES, P, 64)
    sq = (sin * scale).reshape(N_STILES, P, 64)
    ck = cos.reshape(N_STILES, P, 64)
    sk = sin.reshape(N_STILES, P, 64)
    ident = np.eye(P, dtype=np.float32)

    perm = np.concatenate([np.arange(0, HEAD_DIM, 2), np.arange(1, HEAD_DIM, 2)])
    in_maps = []
    for c in range(N_CORES):
        cols_pk = np.concatenate([c * 256 + h * 128 + perm for h in range(HPC)])
        sl = slice(c * 256, (c + 1) * 256)
        in_maps.append({
            "hs": hs,
            "wq": np.ascontiguousarray(Wq.T[:, cols_pk]),
            "wk": np.ascontiguousarray(Wk.T[:, cols_pk]),
            "wv": np.ascontiguousarray(Wv.T[:, sl]),
            "wo": np.ascontiguousarray(Wo.T[sl, :]),
            "bq": np.ascontiguousarray(bq[cols_pk]).reshape(1, DLOC),
            "bk": np.ascontiguousarray(bk[cols_pk]).reshape(1, DLOC),
            "bv": np.ascontiguousarray(bv[sl]).reshape(1, DLOC),
            "bo": np.ascontiguousarray((bo / N_CORES).reshape(N_HTILES, P).T),
            "cq": cq, "sq": sq, "ck": ck, "sk": sk,
            "ident": ident,
        })
    return in_maps


def _maybe_enable_ldw_opt():
    """Experimental: the stock compile pipeline passes --enable-ldw-opt=false;
    flipping it on lets walrus pipeline LDWEIGHTS behind matmuls. Controlled
    by LDW_OPT=1 env; correctness is re-checked by the caller."""
    import os
    if os.environ.get("LDW_OPT", "0") != "1":
        return
    import concourse.bass_utils as bu
    if getattr(bu, "_ldw_patched", False):
        return
    orig = bu.bir_verify_and_optimise

    def patched(tmpdir, inp="bir.json", outp="file.neff", arch=None, **kw):
        import unittest.mock as um
        real_run = bu.run_command

        def run2(argv, **kwargs):
            argv = [a.replace("--enable-ldw-opt=false", "--enable-ldw-opt=true")
                    for a in argv]
            return real_run(argv, **kwargs)

        with um.patch.object(bu, "run_command", run2):
            return orig(tmpdir, inp, outp, arch, **kw)

    bu.bir_verify_and_optimise = patched
    # bass2jax binds its own reference
    import concourse.bass2jax as b2j
    if hasattr(b2j, "bir_verify_and_optimise"):
        b2j.bir_verify_and_optimise = patched
    bu._ldw_patched = True


def run(inputs, trace=False, mm="bf16"):
    _maybe_enable_ldw_opt()
    from concourse.bass_utils import run_bass_kernel_spmd
    key = mm
    if key not in _CACHE:
        _CACHE[key] = _build_program(mm)
    nc = _CACHE[key]
    in_maps = _host_inputs(inputs)
    res = run_bass_kernel_spmd(nc, in_maps, core_ids=list(range(N_CORES)),
                               trace=trace)
    acc = np.zeros((HID, SEQ), np.float64)
    for r in res.results:
        acc += r["pout"].astype(np.float64)
    out = acc.T.astype(np.float32).reshape(1, SEQ, HID)
    return out, res


def kernel(**inputs) -> np.ndarray:
    out, _ = run(inputs, trace=False)
    return out


# revision 31
# speedup vs baseline: 2.2245x; 1.0142x over previous
"""Trainium2 Bass kernel for MiniGPT4 CAM sparse attention.

Sharding: tensor-parallel over 16 heads -> 2 heads per core (8 cores).
Each core: q/k/v projections for its 2 heads (k/v only at the CAM-allowed
kv positions: [0,410) + [3072,4096)), RoPE, attention with the softmax
denominator folded into an augmented-V matmul (ones column), and a
row-sharded output projection producing a partial [2048, 4096] final^T.
Partials are summed on the host (gather/unshard step).

The three phases are interleaved into one pipeline: the CAM kv subtiles
are projected first, so attention for a query block starts as soon as its
q^T columns exist, and the output projection follows per query block.
Matmuls run in bf16 (full-rate PE); accumulation fp32 in PSUM.
"""

import math
import numpy as np

HID = 2048
HEADS = 16
HEAD_DIM = 128
SEQ = 4096
N_CORES = 8
HPC = HEADS // N_CORES          # heads per core = 2
DLOC = HPC * HEAD_DIM           # 256 local head dims
START = math.ceil(0.1 * SEQ)    # 410
RECENT = math.ceil(0.25 * SEQ)  # 1024
KV_HI = SEQ - RECENT            # 3072
NKV = START + RECENT            # 1434
SHIFT = 15.0                    # global exp shift (cancels in softmax)

P = 128
N_STILES = SEQ // P             # 32 seq subtiles
N_HTILES = HID // P             # 16 hidden tiles
QB = 512                        # q block (free dim of scores matmuls)
N_QB = SEQ // QB                # 8

# kv subtiles: (seq_subtile_index, rows_used) covering [0,410) + [3072,4096)
KV_STILES = [(0, 128), (1, 128), (2, 128), (3, 26)] + [(24 + i, 128) for i in range(8)]
N_KVT = len(KV_STILES)          # 12
KV_COL = np.cumsum([0] + [r for _, r in KV_STILES]).tolist()
NKV_PAD = 1440

_CACHE = {}


def _build_program(mm="bf16"):
    import concourse.bass as bass
    import concourse.bacc as bacc
    import concourse.mybir as mybir
    from concourse.tile import TileContext
    from contextlib import ExitStack

    fp32 = mybir.dt.float32
    bf16 = mybir.dt.bfloat16
    AF = mybir.ActivationFunctionType
    ALU = mybir.AluOpType

    nc = bacc.Bacc()

    hs_d = nc.declare_dram_parameter("hs", [SEQ, HID], fp32, isOutput=False)
    wq_d = nc.declare_dram_parameter("wq", [HID, DLOC], fp32, isOutput=False)
    wk_d = nc.declare_dram_parameter("wk", [HID, DLOC], fp32, isOutput=False)
    wv_d = nc.declare_dram_parameter("wv", [HID, DLOC], fp32, isOutput=False)
    wo_d = nc.declare_dram_parameter("wo", [DLOC, HID], fp32, isOutput=False)
    bq_d = nc.declare_dram_parameter("bq", [1, DLOC], fp32, isOutput=False)
    bk_d = nc.declare_dram_parameter("bk", [1, DLOC], fp32, isOutput=False)
    bv_d = nc.declare_dram_parameter("bv", [1, DLOC], fp32, isOutput=False)
    bo_d = nc.declare_dram_parameter("bo", [P, N_HTILES], fp32, isOutput=False)
    cq_d = nc.declare_dram_parameter("cq", [N_STILES, P, 64], fp32, isOutput=False)
    sq_d = nc.declare_dram_parameter("sq", [N_STILES, P, 64], fp32, isOutput=False)
    ck_d = nc.declare_dram_parameter("ck", [N_STILES, P, 64], fp32, isOutput=False)
    sk_d = nc.declare_dram_parameter("sk", [N_STILES, P, 64], fp32, isOutput=False)
    id_d = nc.declare_dram_parameter("ident", [P, P], fp32, isOutput=False)
    out_d = nc.declare_dram_parameter("pout", [HID, SEQ], fp32, isOutput=True)

    with TileContext(nc) as tc, ExitStack() as top:
        const = top.enter_context(tc.tile_pool(name="const", bufs=1))
        persist = top.enter_context(tc.tile_pool(name="persist", bufs=1))
        work = top.enter_context(tc.tile_pool(name="work", bufs=1))
        psum = top.enter_context(tc.tile_pool(name="psum", bufs=1, space="PSUM"))

        # ---- constants (bf16 matmul operands via SWDGE cast DMA) ----
        # fused [wq | wk] per h-tile so kv subtiles project q and k together
        wqk_sb = const.tile([P, N_HTILES, 2 * DLOC], bf16)
        nc.gpsimd.dma_start(
            wqk_sb[:].rearrange("p t (x d) -> p t x d", x=2)[:, :, 0, :],
            wq_d[:].rearrange("(t p) d -> p t d", p=P))
        nc.gpsimd.dma_start(
            wqk_sb[:].rearrange("p t (x d) -> p t x d", x=2)[:, :, 1, :],
            wk_d[:].rearrange("(t p) d -> p t d", p=P))
        wv_sb = const.tile([P, N_HTILES, DLOC], bf16)
        nc.gpsimd.dma_start(wv_sb[:], wv_d[:].rearrange("(t p) d -> p t d", p=P))
        wo_sb = const.tile([P, HPC, HID], bf16)
        nc.gpsimd.dma_start(wo_sb[:], wo_d[:].rearrange("(t p) e -> p t e", p=P))

        bqk_sb = const.tile([1, 2 * DLOC], bf16)
        nc.gpsimd.dma_start(bqk_sb[:, 0:DLOC], bq_d[:])
        nc.gpsimd.dma_start(bqk_sb[:, DLOC:2 * DLOC], bk_d[:])
        bv_sb = const.tile([1, DLOC], bf16)
        nc.gpsimd.dma_start(bv_sb[:], bv_d[:])
        boc_sb = const.tile([P, N_HTILES], fp32)
        nc.sync.dma_start(boc_sb[:], bo_d[:])
        ident_sb = const.tile([P, P], bf16)
        nc.gpsimd.dma_start(ident_sb[:], id_d[:])
        cq_sb = const.tile([P, N_STILES, 64], fp32)
        sq_sb = const.tile([P, N_STILES, 64], fp32)
        nc.sync.dma_start(cq_sb[:], cq_d[:].rearrange("t p d -> p t d"))
        nc.sync.dma_start(sq_sb[:], sq_d[:].rearrange("t p d -> p t d"))
        ones_sb = const.tile([1, QB], bf16)
        nc.gpsimd.memset(ones_sb[:], 1.0)
        onescol_sb = const.tile([P, 1], bf16)
        nc.gpsimd.memset(onescol_sb[:], 1.0)
        shift_sb = const.tile([P, 1], fp32)
        nc.gpsimd.memset(shift_sb[:], -SHIFT)

        # ---- persistent activations ----
        qT_sb = persist.tile([P, HPC, SEQ], bf16)
        kT_sb = persist.tile([P, HPC, NKV_PAD], bf16)
        vA_sb = persist.tile([P, HPC, N_KVT, P], bf16)
        oT_sb = persist.tile([P, HPC, SEQ], bf16)

        kv_of_stile = {st: (i, rows) for i, (st, rows) in enumerate(KV_STILES)}

        # ---------------- phase A worker: one seq subtile ----------------
        def do_subtile(st):
            is_kv = st in kv_of_stile
            hs_t = work.tile([P, HID], bf16, tag="hs", bufs=3, name=f"hs{st}")
            nc.gpsimd.dma_start(hs_t[:], hs_d[st * P:(st + 1) * P, :])

            hsT = work.tile([P, N_HTILES, P], bf16, tag="hsT", bufs=3,
                            name=f"hsT{st}")
            for g in range(N_HTILES // 4):
                ps = psum.tile([P, 4 * P], bf16, tag="tp", bufs=1, name=f"tp{st}_{g}")
                for j in range(4):
                    ht = g * 4 + j
                    nc.tensor.transpose(ps[:, j * P:(j + 1) * P],
                                        hs_t[:, ht * P:(ht + 1) * P], ident_sb[:])
                nc.scalar.copy(
                    hsT[:, g * 4:(g + 1) * 4, :].rearrange("p a b -> p (a b)"),
                    ps[:])

            # q (and on kv subtiles also k) projection, fused rhs [wq|wk]
            nw = 2 * DLOC if is_kv else DLOC
            qkp = psum.tile([P, 2 * DLOC], fp32, tag="qp", bufs=2, name=f"qk{st}")
            nc.tensor.matmul(qkp[:, 0:nw], ones_sb[:, 0:P], bqk_sb[:, 0:nw],
                             start=True, stop=False)
            for ht in range(N_HTILES):
                nc.tensor.matmul(qkp[:, 0:nw], hsT[:, ht, :], wqk_sb[:, ht, 0:nw],
                                 start=False, stop=(ht == N_HTILES - 1))

            def rope(dst, src, cos_ap, sin_ap, rows=P):
                sr = src.rearrange("p (h x d) -> p h x d", h=HPC, x=2)
                dr = dst.rearrange("p (h x d) -> p h x d", h=HPC, x=2)
                x1, x2 = sr[:, :, 0, :], sr[:, :, 1, :]
                o1, o2 = dr[:, :, 0, :], dr[:, :, 1, :]
                cb = cos_ap.rearrange("p (o d) -> p o d", o=1).broadcast_to([rows, HPC, 64])
                sbb = sin_ap.rearrange("p (o d) -> p o d", o=1).broadcast_to([rows, HPC, 64])
                tmp = work.tile([P, P], fp32, tag="rtmp", bufs=3, name=f"rt{st}")
                tr = tmp[0:rows, :].rearrange("p (h d) -> p h d", h=HPC)
                nc.vector.tensor_tensor(o1, x1, cb, ALU.mult)
                nc.vector.tensor_tensor(tr, x2, sbb, ALU.mult)
                nc.vector.tensor_tensor(o1, o1, tr, ALU.subtract)
                nc.vector.tensor_tensor(o2, x1, sbb, ALU.mult)
                nc.vector.tensor_tensor(tr, x2, cb, ALU.mult)
                nc.vector.tensor_tensor(o2, o2, tr, ALU.add)

            qr = work.tile([P, DLOC], bf16, tag="qr", bufs=3, name=f"qr{st}")
            rope(qr[:], qkp[:, 0:DLOC], cq_sb[:, st, :], sq_sb[:, st, :])
            # transpose roped q into qT (both heads -> one psum, one copy)
            pq = psum.tile([P, 2 * P], bf16, tag="tq", bufs=1, name=f"pq{st}")
            for h in range(HPC):
                nc.tensor.transpose(pq[:, h * P:(h + 1) * P],
                                    qr[:, h * P:(h + 1) * P], ident_sb[:])
            nc.vector.tensor_copy(
                qT_sb[:, :, st * P:(st + 1) * P],
                pq[:].rearrange("p (a b) -> p a b", a=HPC))

            if is_kv:
                ti, rows = kv_of_stile[st]
                col = KV_COL[ti]
                ck_t = work.tile([P, 64], fp32, tag="ck", bufs=2, name=f"ck{st}")
                sk_t = work.tile([P, 64], fp32, tag="sk", bufs=2, name=f"sk{st}")
                nc.sync.dma_start(ck_t[:rows, :], ck_d[st, 0:rows, :])
                nc.sync.dma_start(sk_t[:rows, :], sk_d[st, 0:rows, :])

                kr = work.tile([P, DLOC], bf16, tag="qr", bufs=3, name=f"kr{st}")
                rope(kr[0:rows, :], qkp[0:rows, DLOC:2 * DLOC],
                     ck_t[0:rows, :], sk_t[0:rows, :], rows=rows)
                for h in range(HPC):
                    pk = psum.tile([P, P], bf16, tag="tq", bufs=1,
                                   name=f"pk{st}_{h}")
                    nc.tensor.transpose(pk[:, 0:rows],
                                        kr[0:rows, h * P:(h + 1) * P],
                                        ident_sb[0:rows, 0:rows])
                    nc.vector.tensor_copy(kT_sb[:, h, col:col + rows],
                                          pk[:, 0:rows])

                vp = psum.tile([P, 2 * DLOC], fp32, tag="qp", bufs=2, name=f"v{st}")
                nc.tensor.matmul(vp[0:rows, 0:DLOC], ones_sb[:, 0:rows], bv_sb[:],
                                 start=True, stop=False)
                for ht in range(N_HTILES):
                    nc.tensor.matmul(vp[0:rows, 0:DLOC], hsT[:, ht, 0:rows],
                                     wv_sb[:, ht, :],
                                     start=False, stop=(ht == N_HTILES - 1))
                for h in range(HPC):
                    nc.vector.tensor_copy(vA_sb[0:rows, h, ti, 0:P],
                                          vp[0:rows, h * P:(h + 1) * P])

        # ---------------- phase B workers: attention for one q block ------
        ex_tiles = {}

        def do_scores(qb):
            for h in range(HPC):
                ex = work.tile([P, N_KVT, QB], bf16, tag="exp", bufs=4,
                               name=f"ex{qb}_{h}")
                ex_tiles[(qb, h)] = ex
                for ti, (st, rows) in enumerate(KV_STILES):
                    col = KV_COL[ti]
                    sp = psum.tile([P, QB], fp32, tag="sc", bufs=2,
                                   name=f"sc{qb}_{h}_{ti}")
                    nc.tensor.matmul(sp[0:rows, :],
                                     kT_sb[:, h, col:col + rows],
                                     qT_sb[:, h, qb * QB:(qb + 1) * QB],
                                     start=True, stop=True)
                    nc.scalar.activation(ex[0:rows, ti, :], sp[0:rows, :],
                                         AF.Exp, bias=shift_sb[0:rows, :],
                                         scale=1.0)

        # kv tiles with full 128 rows, for the denominator pair-tree
        FULL_TIS = [ti for ti, (st, r) in enumerate(KV_STILES) if r == P]
        SHORT_TI = [ti for ti, (st, r) in enumerate(KV_STILES) if r != P][0]

        def do_attnv(qb):
            for h in range(HPC):
                ex = ex_tiles.pop((qb, h))
                # O^T accumulation [128d, 512q] with N=512 matmuls
                op = psum.tile([P, QB], fp32, tag="ov", bufs=2,
                               name=f"ov{qb}_{h}")
                dn = psum.tile([1, QB], fp32, tag="tq", bufs=1,
                               name=f"dn{qb}_{h}")
                for ti, (st, rows) in enumerate(KV_STILES):
                    nc.tensor.matmul(op[:],
                                     vA_sb[0:rows, h, ti, :],
                                     ex[0:rows, ti, :],
                                     start=(ti == 0), stop=(ti == N_KVT - 1))
                    nc.tensor.matmul(dn[:],
                                     onescol_sb[0:rows, :],
                                     ex[0:rows, ti, :],
                                     start=(ti == 0), stop=(ti == N_KVT - 1))
                recip = work.tile([1, QB], fp32, tag="recip", bufs=3,
                                  name=f"rc{qb}_{h}")
                nc.vector.reciprocal(recip[:], dn[:])
                rb = work.tile([P, QB], fp32, tag="rb", bufs=3,
                               name=f"rb{qb}_{h}")
                nc.gpsimd.partition_broadcast(rb[:], recip[:])
                nc.vector.tensor_tensor(oT_sb[:, h, qb * QB:(qb + 1) * QB],
                                        op[:], rb[:], ALU.mult)

        # ---------------- phase C worker: out-proj for one q block --------
        def do_oproj(qb):
            for et in range(N_HTILES):
                fp = psum.tile([P, QB], fp32, tag="qp", bufs=2,
                               name=f"fp{qb}_{et}")
                for h in range(HPC):
                    nc.tensor.matmul(fp[:],
                                     wo_sb[:, h, et * P:(et + 1) * P],
                                     oT_sb[:, h, qb * QB:(qb + 1) * QB],
                                     start=(h == 0), stop=(h == HPC - 1))
                stg = work.tile([P, QB], fp32, tag="stage", bufs=4,
                                name=f"st{qb}_{et}")
                # copy PSUM->SBUF with the (per-partition) bo bias folded in
                nc.vector.tensor_scalar_add(stg[:], fp[:],
                                            boc_sb[:, et:et + 1])
                nc.sync.dma_start(
                    out_d[et * P:(et + 1) * P, qb * QB:(qb + 1) * QB], stg[:])

        # ---------------- interleaved pipeline ----------------
        # kv subtiles first; then per q block: scores -> (weave in remaining
        # subtiles so the PE never head-of-line-blocks on ACT's exp) ->
        # attention@V -> more subtiles -> out-projection.
        from collections import deque
        for st in [st for st, _ in KV_STILES]:
            do_subtile(st)
        pending = deque(range(4, 24))
        emitted = set(st for st, _ in KV_STILES)

        def emit_a(n):
            for _ in range(n):
                if pending:
                    st = pending.popleft()
                    do_subtile(st)
                    emitted.add(st)

        for qb in (0, 6, 7, 1, 2, 3, 4, 5):
            while not all(s in emitted for s in range(qb * 4, qb * 4 + 4)):
                emit_a(1)
            do_scores(qb)
            emit_a(1)
            do_attnv(qb)
            emit_a(2)
            do_oproj(qb)

    nc.finalize()
    return nc


def _host_inputs(inputs):
    hs = np.ascontiguousarray(np.asarray(inputs["hidden_states"], np.float32).reshape(SEQ, HID))
    Wq = np.asarray(inputs["Wq"], np.float32)
    Wk = np.asarray(inputs["Wk"], np.float32)
    Wv = np.asarray(inputs["Wv"], np.float32)
    Wo = np.asarray(inputs["Wo"], np.float32)
    bq = np.asarray(inputs["bq"], np.float32)
    bk = np.asarray(inputs["bk"], np.float32)
    bv = np.asarray(inputs["bv"], np.float32)
    bo = np.asarray(inputs["bo"], np.float32)

    theta = 1.0 / (10000.0 ** (np.arange(0, HEAD_DIM, 2, dtype=np.float32) / HEAD_DIM))
    sinusoid = np.arange(SEQ, dtype=np.float32)[:, None] * theta[None, :]
    sin = np.sin(sinusoid).astype(np.float32)
    cos = np.cos(sinusoid).astype(np.float32)
    scale = np.float32(1.0 / math.sqrt(HEAD_DIM))
    cq = (cos * scale).reshape(N_STILES, P, 64)
    sq = (sin * scale).reshape(N_STILES, P, 64)
    ck = cos.reshape(N_STILES, P, 64)
    sk = sin.reshape(N_STILES, P, 64)
    ident = np.eye(P, dtype=np.float32)

    perm = np.concatenate([np.arange(0, HEAD_DIM, 2), np.arange(1, HEAD_DIM, 2)])
    in_maps = []
    for c in range(N_CORES):
        cols_pk = np.concatenate([c * 256 + h * 128 + perm for h in range(HPC)])
        sl = slice(c * 256, (c + 1) * 256)
        in_maps.append({
            "hs": hs,
            "wq": np.ascontiguousarray(Wq.T[:, cols_pk]),
            "wk": np.ascontiguousarray(Wk.T[:, cols_pk]),
            "wv": np.ascontiguousarray(Wv.T[:, sl]),
            "wo": np.ascontiguousarray(Wo.T[sl, :]),
            "bq": np.ascontiguousarray(bq[cols_pk]).reshape(1, DLOC),
            "bk": np.ascontiguousarray(bk[cols_pk]).reshape(1, DLOC),
            "bv": np.ascontiguousarray(bv[sl]).reshape(1, DLOC),
            "bo": np.ascontiguousarray((bo / N_CORES).reshape(N_HTILES, P).T),
            "cq": cq, "sq": sq, "ck": ck, "sk": sk,
            "ident": ident,
        })
    return in_maps


def _maybe_enable_ldw_opt():
    """Experimental: the stock compile pipeline passes --enable-ldw-opt=false;
    flipping it on lets walrus pipeline LDWEIGHTS behind matmuls. Controlled
    by LDW_OPT=1 env; correctness is re-checked by the caller."""
    import os
    if os.environ.get("LDW_OPT", "0") != "1":
        return
    import concourse.bass_utils as bu
    if getattr(bu, "_ldw_patched", False):
        return
    orig = bu.bir_verify_and_optimise

    def patched(tmpdir, inp="bir.json", outp="file.neff", arch=None, **kw):
        import unittest.mock as um
        real_run = bu.run_command

        def run2(argv, **kwargs):
            argv = [a.replace("--enable-ldw-opt=false", "--enable-ldw-opt=true")
                    for a in argv]
            return real_run(argv, **kwargs)

        with um.patch.object(bu, "run_command", run2):
            return orig(tmpdir, inp, outp, arch, **kw)

    bu.bir_verify_and_optimise = patched
    # bass2jax binds its own reference
    import concourse.bass2jax as b2j
    if hasattr(b2j, "bir_verify_and_optimise"):
        b2j.bir_verify_and_optimise = patched
    bu._ldw_patched = True


def run(inputs, trace=False, mm="bf16"):
    _maybe_enable_ldw_opt()
    from concourse.bass_utils import run_bass_kernel_spmd
    key = mm
    if key not in _CACHE:
        _CACHE[key] = _build_program(mm)
    nc = _CACHE[key]
    in_maps = _host_inputs(inputs)
    res = run_bass_kernel_spmd(nc, in_maps, core_ids=list(range(N_CORES)),
                               trace=trace)
    acc = np.zeros((HID, SEQ), np.float64)
    for r in res.results:
        acc += r["pout"].astype(np.float64)
    out = acc.T.astype(np.float32).reshape(1, SEQ, HID)
    return out, res


def kernel(**inputs) -> np.ndarray:
    out, _ = run(inputs, trace=False)
    return out
